# revision 1
# baseline (speedup 1.0000x reference)
"""Trainium2 Bass kernel for the OFPenalty eigenvalue-penalty loss.

Math (per sample b of 256):
  W = x[b] reshaped [C=2048, N=49];  G = W^T W  (49x49 Gram matrix)
  run1: x9 = G^9 x0 (power iteration, normalization deferred - scale
        invariant), largest = Rayleigh(G, x9) = x9^T G x9 / x9^T x9
  run2: B = G - largest*I, u9 = B^9 x1 (x1 = scaled x9),
        tmp = Rayleigh(B, u9); smallest = tmp + largest
  penalty = (largest/smallest - 1)^2 ; output = mean over batch.

Sharding: pure data parallel, 32 samples per core on 8 cores.  Samples
are processed in pairs packed block-diagonally: sample 2p lives on
partitions 0:49, sample 2p+1 on partitions 64:113 (the gap keeps every
compute-engine access 32-partition aligned).  Scalings by powers of two
(exact) keep the unnormalized power iterates inside fp32 range.
Rayleigh numerators/denominators are columnwise dot products: masked
elementwise multiply on VectorE, then a ones-vector matmul reduces over
partitions, leaving per-sample scalars in free-dim rows.
"""

import os
import sys
from contextlib import ExitStack

import numpy as np

for _p in ("/opt/trn_rl_repo",):
    if os.path.isdir(_p) and _p not in sys.path:
        sys.path.insert(0, _p)

import concourse.bass as bass  # noqa: E402
import concourse.tile as tile  # noqa: E402
from concourse import bacc, mybir  # noqa: E402
from concourse.bass_utils import run_bass_kernel_spmd  # noqa: E402

F32 = mybir.dt.float32
I32 = mybir.dt.int32
ALU = mybir.AluOpType

B, C, N = 256, 2048, 49
NCORES = 8
BS = B // NCORES  # 32 samples per core
NPAIR = BS // 2  # 16 pairs
KT = C // 128  # 16 contraction tiles
PG = 128  # gapped pair-vector space: blocks at [0:49], [64:113]
B1 = 64  # partition base of the second sample in a pair
S52 = float(2.0**-52)  # rescale before Rayleigh products
S102 = float(2.0**-102)  # rescale x9 -> x1 (run2 warm start)
NITER = 9


def _chain_waves(nc, stats, v0, nsteps, vpool, pspool, label, lamv=None):
    """Apply per-pair matrices nsteps times to all NPAIR columns in lockstep.

    Each wave issues one matvec per pair into columns of a shared PSUM
    tile; one batched fixup/copy feeds the next wave.  With lamv given,
    the matrices act as shifted A - lam*I without materializing them:
    the inter-wave step computes nxt = psum - lamv*cur (the lamv*cur
    product is issued before the matvecs so it hides under them).

    Returns (last_sbuf, last_psum, last_t): the input vector of the
    final wave (SBUF), the final wave's raw A*v PSUM, and the final
    wave's lamv*cur product (None when lamv is None).
    """
    cur = v0
    psw = None
    last_sbuf = None
    t = None
    for i in range(nsteps):
        psw = pspool.tile([PG, NPAIR], F32, tag="mvw", name=f"mvw_{label}{i}")
        if lamv is not None:
            t = vpool.tile([PG, NPAIR], F32, tag="vt", name=f"vt_{label}{i}")
            nc.vector.tensor_mul(t[:], lamv, cur)
        for p in range(NPAIR):
            nc.tensor.matmul(
                psw[:, p : p + 1], stats[p], cur[:, p : p + 1],
                start=True, stop=True,
            )
        if i < nsteps - 1:
            nxt = vpool.tile([PG, NPAIR], F32, tag="vw", name=f"vw_{label}{i}")
            if lamv is not None:
                nc.vector.tensor_sub(nxt[:], psw[:], t[:])
            else:
                nc.vector.tensor_copy(nxt[:], psw[:])
            last_sbuf = nxt
            cur = nxt[:]
    return last_sbuf, psw, t


def _emit(tc, x, x0, pen, repeat=1):
    nc = tc.nc
    ctx = ExitStack()
    with ctx:
        const = ctx.enter_context(tc.tile_pool(name="const", bufs=1))
        xpool = ctx.enter_context(tc.tile_pool(name="xt", bufs=4))
        vpool = ctx.enter_context(tc.tile_pool(name="vec", bufs=3))
        ps_ata = ctx.enter_context(tc.tile_pool(name="ps_ata", bufs=4, space="PSUM"))
        ps_mv = ctx.enter_context(tc.tile_pool(name="ps_mv", bufs=3, space="PSUM"))
        ps_msc = ctx.enter_context(tc.tile_pool(name="ps_msc", bufs=1, space="PSUM"))

        # ---- constants -------------------------------------------------
        # x0 columns: X0[0:49, p] = x0[2p], X0[64:113, p] = x0[2p+1]
        X0 = const.tile([PG, NPAIR], F32)
        nc.gpsimd.memset(X0[:], 0.0)
        x0r = x0.rearrange("(p two) j -> two j p", two=2)
        nc.sync.dma_start(X0[0:N, :], x0r[0])
        nc.sync.dma_start(X0[B1 : B1 + N, :], x0r[1])

        # identity mask (used to build B = A - lambda*I)
        DIAG = const.tile([PG, PG], F32)
        nc.gpsimd.memset(DIAG[:], 0.0)
        nc.gpsimd.affine_select(
            out=DIAG[:],
            in_=DIAG[:],
            compare_op=ALU.not_equal,
            fill=1.0,
            base=0,
            pattern=[[-1, PG]],
            channel_multiplier=1,
        )

        # block-ownership row masks: CM0 = 1 on partitions of sample 0's
        # block (cols 0:49), CM1 on sample 1's block (cols 64:113)
        CM0 = const.tile([1, PG], F32)
        nc.gpsimd.memset(CM0[:], 0.0)
        nc.gpsimd.memset(CM0[:, 0:N], 1.0)
        CM1 = const.tile([1, PG], F32)
        nc.gpsimd.memset(CM1[:], 0.0)
        nc.gpsimd.memset(CM1[:, B1 : B1 + N], 1.0)

        ONE128 = const.tile([PG, 1], F32)
        nc.gpsimd.memset(ONE128[:], 1.0)

        # ---- persistent intermediates ---------------------------------
        X9M = const.tile([PG, BS], F32)  # block-masked x9, col per sample
        WF1 = const.tile([PG, NPAIR], F32)  # w columns, one per pair
        XF1 = const.tile([PG, NPAIR], F32)  # scaled x9 columns, one per pair
        U9M = const.tile([PG, BS], F32)
        WF2 = const.tile([PG, NPAIR], F32)
        XF2 = const.tile([PG, NPAIR], F32)
        X1A = const.tile([PG, NPAIR], F32)  # run2 warm starts
        LAMV = const.tile([PG, NPAIR], F32)  # lambda per partition
        nc.gpsimd.memset(X9M[:], 0.0)
        nc.gpsimd.memset(U9M[:], 0.0)
        Aall = const.tile([PG, NPAIR, PG], F32)  # blockdiag Gram per pair
        Ball = const.tile([PG, NPAIR, PG], F32)  # shifted matrices
        lamI = const.tile([PG, NPAIR, PG], F32)

        for _rep in range(repeat):
            # ---- phase 1: Gram matrices -----------------------------------
            # Partition q holds c-rows {512b + 4q + r : r<4}, sample-major in
            # SBUF: 784B-contiguous DMA descriptors (>=512B keeps DMA at full
            # bandwidth) AND contiguous [128, 49] matmul stationaries.
            # Sample 1's Gram accumulates into psum partitions 64:113 (PE
            # column-group 64), so downstream block layout is unchanged.
            xrs = x.rearrange(
                "(p two) (b q r) j -> p two q b (r j)", two=2, b=4, q=128, r=4
            )
            nc.gpsimd.memset(Aall[:], 0.0)
            As = []
            for p in range(NPAIR):
                xt = xpool.tile([128, 2, KT * N], F32, tag="xt", name=f"xt{p}")
                for s in range(2):
                    eng = nc.sync if s == 0 else nc.scalar
                    eng.dma_start(
                        xt[:, s, :].rearrange("q (b m) -> q b m", b=4),
                        xrs[p, s],
                    )
                # interleave the two samples' accumulation groups: they
                # occupy PE column-groups 0 and 64 (and separate PSUM
                # banks), so adjacent matmuls can overlap in the array
                psa = ps_ata.tile([PG, N], F32, tag="ata", name=f"ata{p}a")
                psb = ps_ata.tile([PG, N], F32, tag="ata", name=f"ata{p}b")
                for k in range(KT):
                    for s in range(2):
                        pst = psa if s == 0 else psb
                        ob = 0 if s == 0 else B1
                        wk = xt[:, s, k * N : (k + 1) * N]
                        nc.tensor.matmul(
                            pst[ob : ob + N, :],
                            wk,
                            wk,
                            start=(k == 0),
                            stop=(k == KT - 1),
                        )
                A = Aall[:, p, :]
                nc.scalar.copy(A[0:N, 0:N], psa[0:N, :])
                nc.scalar.copy(A[B1 : B1 + N, B1 : B1 + N], psb[B1 : B1 + N, :])
                As.append(A)

            # ---- phase 2: run1 chains (wave-major across pairs) -----------
            # 10 waves: wave 9's input x9 (unscaled SBUF) and wave 10's
            # output w = A*x9 (raw PSUM) come out of the same chain.
            x9u, psw1, _ = _chain_waves(nc, As, X0[:], NITER + 1, vpool, ps_mv, "a")
            x9e = X9M.rearrange("q (p j) -> q p j", j=2)
            nc.vector.tensor_scalar(WF1[:], psw1[:], S52, None, op0=ALU.mult)
            nc.vector.tensor_scalar(XF1[:], x9u[:], S52, None, op0=ALU.mult)
            nc.vector.tensor_scalar(
                x9e[0:N, :, 0], x9u[0:N, :], S52, None, op0=ALU.mult
            )
            nc.vector.tensor_scalar(
                x9e[B1 : B1 + N, :, 1], x9u[B1 : B1 + N, :], S52, None, op0=ALU.mult
            )
            nc.vector.tensor_scalar(X1A[:], x9u[:], S102, None, op0=ALU.mult)

            def rayleigh_rows(X9M_, WF_, XF_, ndrow, label):
                # T[:, 0:32] = X9M * w(dup per sample); T[:, 32:64] = X9M * x9(dup)
                T = const.tile([PG, 2 * BS], F32, name=f"T{label}")
                wdup = WF_[:, :, None].broadcast_to([PG, NPAIR, 2])
                xdup = XF_[:, :, None].broadcast_to([PG, NPAIR, 2])
                nc.vector.tensor_mul(T[:, 0:BS], X9M_[:], wdup)
                nc.vector.tensor_mul(T[:, BS : 2 * BS], X9M_[:], xdup)
                pnd = ps_msc.tile([1, 2 * BS], F32, tag="msc", name=f"pnd{label}")
                nc.tensor.matmul(pnd[:], ONE128[:], T[:], start=True, stop=True)
                nc.scalar.copy(ndrow[:], pnd[:])

            # ---- Rayleigh 1 (num/den rows) + lambda broadcast -------------
            ND1 = const.tile([1, 2 * BS], F32)  # [num row | den row]
            rayleigh_rows(X9M, WF1, XF1, ND1, "r1")
            LAMR = const.tile([1, BS], F32)
            RDR = const.tile([1, BS], F32)
            nc.vector.reciprocal(RDR[:], ND1[:, BS : 2 * BS])
            nc.vector.tensor_mul(LAMR[:], ND1[:, 0:BS], RDR[:])

            # LAMV[q, p] = lambda of the sample owning partition q in pair p,
            # as two accumulating rank-1 matmuls: CM0^T lam_even + CM1^T lam_odd
            lam2 = LAMR.rearrange("o (p h) -> o p h", h=2)
            psl = ps_msc.tile([PG, NPAIR], F32, tag="msc", name="psl")
            nc.tensor.matmul(psl[:], CM0[:], lam2[:, :, 0], start=True, stop=False)
            nc.tensor.matmul(psl[:], CM1[:], lam2[:, :, 1], start=False, stop=True)
            nc.scalar.copy(LAMV[:], psl[:])

            # ---- phase 3: shifted matrices + run2 chains ------------------
            # Ball = Aall - lam*I in two batched VectorE ops (materialized:
            # computing A*v - lam*v per wave instead amplifies the PE's
            # 2-pass fp32 rounding through cancellation; hw err 1.8e-4 vs
            # 2.5e-5 materialized).  Wave 10 gives w2 = B*u9 directly.
            H8 = NPAIR // 2
            db = DIAG[:, None, :].broadcast_to([PG, NPAIR, PG])
            lb = LAMV[:, :, None].broadcast_to([PG, NPAIR, PG])
            nc.vector.tensor_tensor(
                lamI[:, 0:H8], db[:, 0:H8], lb[:, 0:H8], op=ALU.mult
            )
            nc.gpsimd.tensor_tensor(
                lamI[:, H8:NPAIR], db[:, H8:NPAIR], lb[:, H8:NPAIR], op=ALU.mult
            )
            nc.vector.tensor_sub(Ball[:, 0:H8], Aall[:, 0:H8], lamI[:, 0:H8])
            nc.vector.tensor_sub(
                Ball[:, H8:NPAIR], Aall[:, H8:NPAIR], lamI[:, H8:NPAIR]
            )
            Bs = [Ball[:, p, :] for p in range(NPAIR)]
            u9u, psw2, _ = _chain_waves(
                nc, Bs, X1A[:], NITER + 1, vpool, ps_mv, "b"
            )
            u9e = U9M.rearrange("q (p j) -> q p j", j=2)
            nc.vector.tensor_scalar(WF2[:], psw2[:], S52, None, op0=ALU.mult)
            nc.vector.tensor_scalar(XF2[:], u9u[:], S52, None, op0=ALU.mult)
            nc.vector.tensor_scalar(
                u9e[0:N, :, 0], u9u[0:N, :], S52, None, op0=ALU.mult
            )
            nc.vector.tensor_scalar(
                u9e[B1 : B1 + N, :, 1], u9u[B1 : B1 + N, :], S52, None, op0=ALU.mult
            )

            # ---- Rayleigh 2 + penalty (all on free-dim rows) --------------
            ND2 = const.tile([1, 2 * BS], F32)
            rayleigh_rows(U9M, WF2, XF2, ND2, "r2")
            RD2 = const.tile([1, BS], F32)
            TMP = const.tile([1, BS], F32)
            SM = const.tile([1, BS], F32)
            RS = const.tile([1, BS], F32)
            RT = const.tile([1, BS], F32)
            PEN = const.tile([1, BS], F32)
            # (largest/smallest - 1)^2 == (tmp/smallest)^2  (largest-smallest=-tmp)
            nc.vector.reciprocal(RD2[:], ND2[:, BS : 2 * BS])
            nc.vector.tensor_mul(TMP[:], ND2[:, 0:BS], RD2[:])
            nc.vector.tensor_add(SM[:], TMP[:], LAMR[:])
            nc.vector.reciprocal(RS[:], SM[:])
            nc.vector.tensor_mul(RT[:], TMP[:], RS[:])
            nc.vector.tensor_mul(PEN[:], RT[:], RT[:])
            nc.sync.dma_start(pen, PEN[:])


_NC_CACHE = {}


def build_nc(repeat=1):
    if repeat in _NC_CACHE:
        return _NC_CACHE[repeat]
    nc = bacc.Bacc("TRN2", target_bir_lowering=False, debug=False)
    x = nc.dram_tensor("x", [BS, C, N], F32, kind="ExternalInput")
    x0 = nc.dram_tensor("x0", [BS, N], F32, kind="ExternalInput")
    pen = nc.dram_tensor("pen", [BS], F32, kind="ExternalOutput")
    with tile.TileContext(nc) as tc:
        _emit(tc, x.ap(), x0.ap(), pen.ap(), repeat=repeat)
    nc.compile()
    _NC_CACHE[repeat] = nc
    return nc


LAST_RESULTS = None


def kernel(x, x0):
    global LAST_RESULTS
    x = np.ascontiguousarray(np.asarray(x, dtype=np.float32).reshape(B, C, N))
    x0 = np.ascontiguousarray(np.asarray(x0, dtype=np.float32).reshape(B, N))
    nc = build_nc()
    in_maps = [
        {"x": x[i * BS : (i + 1) * BS], "x0": x0[i * BS : (i + 1) * BS]}
        for i in range(NCORES)
    ]
    trace = bool(int(os.environ.get("KERNEL_TRACE", "0")))
    res = run_bass_kernel_spmd(nc, in_maps, list(range(NCORES)), trace=trace)
    LAST_RESULTS = res
    pens = np.concatenate([r["pen"].reshape(-1) for r in res.results])
    return np.float32(pens.sum(dtype=np.float64) / B)



# revision 62
# speedup vs baseline: 1.3260x; 1.3260x over previous
"""Trainium2 Bass kernel for the OFPenalty eigenvalue-penalty loss.

Math (per sample b of 256):
  W = x[b] reshaped [C=2048, N=49];  G = W^T W  (49x49 Gram matrix)
  run1: x9 = G^9 x0 (power iteration, normalization deferred - scale
        invariant), largest = Rayleigh(G, x9) = x9^T G x9 / x9^T x9
  run2: B = G - largest*I applied fused per wave (B v = G v - lam*v),
        u9 = B^9 x1 (x1 = scaled x9), tmp = Rayleigh(B, u9)
  penalty = (tmp/(tmp+largest))^2 ; output = mean over batch.

Layout: pure data parallel, 32 samples per core on 8 cores.  Samples
are packed in pairs block-diagonally: sample 2p on partitions 0:49,
sample 2p+1 on 64:113.  The Gram inputs are converted fp32->fp16 so
the 512 Gram matmuls run at 1 cycle/row instead of 4 (PSUM still
accumulates fp32; rel err stays ~1e-4, far under the 2e-2 gate).

Pipelining: the 16 pairs stream in DMA order; after each group of 4
pairs has its Gram matrices, that group's full eigen-chain (run1,
Rayleigh/lambda, fused run2, penalty) runs while later pairs' DMA +
conversion + Grams continue.  Gram matmuls of later pairs are pumped
into the PE gaps between chain waves.  Only the last group's chain
trails the final DMA.

Rayleigh sums use a block-diagonal-ones stationary so the per-sample
numerator/denominator land broadcast across all partitions in block
layout - lambda feeds the fused run2 with no rank-1 rebroadcast.
Iterate overflow is handled by folding exact powers-of-two scales into
single fixup/scale ops (wave 9 of run1, and the run2 warm start).
"""

import os
import sys
from contextlib import ExitStack

import numpy as np

for _p in ("/opt/trn_rl_repo",):
    if os.path.isdir(_p) and _p not in sys.path:
        sys.path.insert(0, _p)

import concourse.bass as bass  # noqa: E402
import concourse.tile as tile  # noqa: E402
from concourse import bacc, mybir  # noqa: E402
from concourse.bass_utils import run_bass_kernel_spmd  # noqa: E402

F32 = mybir.dt.float32
F16 = mybir.dt.float16
ALU = mybir.AluOpType

B, C, N = 256, 2048, 49
NCORES = 8
BS = B // NCORES  # 32 samples per core
NPAIR = BS // 2  # 16 pairs
KT = C // 128  # 16 contraction tiles
PG = 128  # gapped pair-vector space: blocks at [0:49], [64:113]
B1 = 64  # partition base of the second sample in a pair
NG = 2  # pipeline groups
GP = NPAIR // NG  # pairs per group
NITER = 9
S52 = float(2.0**-52)  # rescale x9 before Rayleigh products
S104 = float(2.0**-104)  # rescale x9 -> x1 (run2 warm start)


def _emit(tc, x, x0, pen):
    nc = tc.nc
    ctx = ExitStack()
    with ctx:
        const = ctx.enter_context(tc.tile_pool(name="const", bufs=1))
        xpool = ctx.enter_context(tc.tile_pool(name="xt", bufs=8))
        hpool = ctx.enter_context(tc.tile_pool(name="xh", bufs=16))
        vpool = ctx.enter_context(tc.tile_pool(name="vec", bufs=6))
        pspool = ctx.enter_context(tc.tile_pool(name="ps", bufs=1, space="PSUM"))

        # ---- constants -------------------------------------------------
        # block-diagonal ones: partition-sum broadcast within each block
        ONESB = const.tile([PG, PG], F32)
        nc.gpsimd.memset(ONESB[:], 0.0)
        nc.gpsimd.memset(ONESB[0:N, 0:N], 1.0)
        nc.gpsimd.memset(ONESB[B1 : B1 + N, B1 : B1 + N], 1.0)
        # x0 columns: X0[0:49, p] = x0[2p], X0[64:113, p] = x0[2p+1].
        # Loaded via a contiguous [32, 49] DMA + PE transpose: a direct
        # strided DMA would burn ~0.7us of 4-byte descriptors on the DMA
        # engines ahead of the x stream.
        X0 = const.tile([PG, NPAIR], F32)
        nc.gpsimd.memset(X0[:], 0.0)
        IDT = const.tile([32, 32], F32)
        nc.gpsimd.memset(IDT[:], 0.0)
        nc.gpsimd.affine_select(
            out=IDT[:], in_=IDT[:], compare_op=ALU.not_equal, fill=1.0,
            base=0, pattern=[[-1, 32]], channel_multiplier=1,
        )
        S0 = const.tile([32, N], F32)
        # block-diagonal Gram matrices (off-block stays zero)
        Aall = const.tile([PG, NPAIR, PG], F32)
        nc.gpsimd.memset(Aall[:], 0.0)
        # squared Grams: G2 drives the B^2 double waves of run2 for every
        # chain and the 6-wave run1 of chain 0; G4 (tail pairs only) gives
        # chain 1 a 4-wave run1 (x9 = G4 G4 G x0)
        G2all = const.tile([PG, NPAIR, PG], F32)
        G4all = const.tile([PG, GP, PG], F32)

        # per-sample DMA view: partition q holds c-rows {512b + 4q + r},
        # 784B-contiguous descriptors (full DMA bandwidth); the (b, r)
        # enumeration of contraction tiles is a permutation of c, which
        # the Gram sum is invariant to.
        xrs = x.rearrange("s (b q r) j -> s q b (r j)", b=4, q=128, r=4)

        # DVE converts at 0.5 cyc/elem (2x mode), Act at 1/1.2GHz, Pool at
        # 1/(1.2GHz*0.6).  DVE takes the early evens (it is chain-free until
        # ~22us), Act the early odds plus the tail pair (it is free when the
        # tail arrives), Pool the middle stretch.
        # tail pairs (14-15) convert on DVE/Act right after chain 0's
        # engine blocks clear - their conv+gram prep is the gate into the
        # tail chain, so it gets the fastest converters.
        FP32_PAIRS = set()
        _CONV = {}
        for _idx in range(2 * NPAIR):
            if _idx <= 15:
                _CONV[_idx] = "dve" if _idx % 2 == 0 else "act"
            elif _idx >= 28:
                _CONV[_idx] = {28: "dve", 29: "pool", 30: "dve", 31: "dve"}[_idx]
            else:
                _CONV[_idx] = "pool"

        def conv_eng(idx):
            return {"dve": nc.vector, "act": nc.scalar, "pool": nc.gpsimd}[
                _CONV[idx]
            ]

        xt_tiles = {}
        xh_tiles = {}
        pair_ps = {}
        pending = []  # deferred half-sample Gram emitters, pumped into chains
        pen_dmas = []

        def emit_dma(p, nsub=1):
            for s in range(2):
                idx = 2 * p + s
                xt = xpool.tile([PG, KT * N], F32, tag="xt", bufs=8, name=f"xt{idx}")
                xv = xt.rearrange("q (b m) -> q b m", b=4)
                for u in range(nsub):
                    lo, hi = u * 4 // nsub, (u + 1) * 4 // nsub
                    nc.sync.dma_start(xv[:, lo:hi], xrs[idx, :, lo:hi])
                xt_tiles[idx] = xt
                if idx == 0:
                    nc.sync.dma_start(S0[:], x0)
                    pst = pspool.tile([N, 32], F32, tag="ray", bufs=1, name="pst")
                    nc.tensor.transpose(pst[:], S0[:], IDT[:])
                    pst_r = pst.rearrange("j (p e) -> e j p", e=2)
                    nc.vector.tensor_copy(X0[0:N, :], pst_r[0])
                    nc.vector.tensor_copy(X0[B1 : B1 + N, :], pst_r[1])

        def _copy(eng, out, in_):
            if eng is nc.scalar:
                eng.copy(out, in_)
            else:
                eng.tensor_copy(out, in_)

        def emit_conv(p):
            halves = 2 if p >= NPAIR - 2 else 1
            for s in range(2):
                idx = 2 * p + s
                xh = hpool.tile([PG, KT * N], F16, tag="xh", name=f"xh{idx}")
                half = KT * N // 2
                for u in range(halves):
                    lo, hi = (u * half, (u + 1) * half) if halves == 2 else (0, KT * N)
                    _copy(conv_eng(idx), xh[:, lo:hi], xt_tiles[idx][:, lo:hi])
                xh_tiles[idx] = xh

        def gram_job(p, s, h=None):
            # h=None: the whole sample (16 tiles, ~0.33us fp16); h=0/1: one
            # half - used for the fp32 tail pairs so their (4x costlier)
            # matmuls pipeline with the per-half sub-DMAs
            def job():
                if s == 0 and h in (None, 0):
                    pair_ps[p] = pspool.tile(
                        [PG, N], F32, tag="gram", bufs=3, name=f"pg{p}"
                    )
                pg_ = pair_ps[p]
                src = xt_tiles[2 * p + s] if p in FP32_PAIRS else xh_tiles[2 * p + s]
                ob = 0 if s == 0 else B1
                ks = range(KT) if h is None else range(8 * h, 8 * h + 8)
                for k in ks:
                    wk = src[:, k * N : (k + 1) * N]
                    nc.tensor.matmul(
                        pg_[ob : ob + N, :], wk, wk,
                        start=(k == 0), stop=(k == KT - 1),
                    )
            return job

        def copy_job(p, s):
            def job():
                pg_ = pair_ps[p]
                if s == 0:
                    nc.scalar.copy(Aall[0:N, p, 0:N], pg_[0:N, :])
                else:
                    nc.scalar.copy(
                        Aall[B1 : B1 + N, p, B1 : B1 + N], pg_[B1 : B1 + N, :]
                    )
            return job

        def sq_job(p, stage, part):
            # stage 0: G2 = A*A, stage 1: G4 = G2*G2; part 0 = PE matmul,
            # part 1 = PSUM->SBUF copy (DVE mid-stream; Act for the tail
            # pairs, since DVE is still busy with chain 0 when they land)
            def job():
                src = Aall[:, p, :] if stage == 0 else G2all[:, p, :]
                dst = G2all[:, p, :] if stage == 0 else G4all[:, p - GP, :]
                if part == 0:
                    ps2 = pspool.tile(
                        [PG, PG], F32, tag="sq", bufs=2, name=f"sq{p}_{stage}"
                    )
                    pair_ps[("sq", p)] = ps2
                    nc.tensor.matmul(ps2[:], src, src, start=True, stop=True)
                elif p >= NPAIR - 2:
                    nc.scalar.copy(dst, pair_ps.pop(("sq", p))[:])
                else:
                    nc.vector.tensor_copy(dst, pair_ps.pop(("sq", p))[:])
            return job

        # Jobs are (pair, thunk).  pump() only feeds jobs whose pair's data
        # lands while chain 0 is running (pairs < PUMP_CUTOFF); later pairs
        # would stall the chain waves on their DMA, so they run in the
        # post-chain-0 engine-idle window instead (via flush).
        PUMP_CUTOFF = NPAIR - 2

        def pump(k=1):
            for _ in range(k):
                if pending and pending[0][0] < PUMP_CUTOFF:
                    pending.pop(0)[1]()

        def flush():
            while pending:
                pending.pop(0)[1]()

        def queue_pair(p, tail=False):
            emit_dma(p, nsub=2 if p in FP32_PAIRS else 1)
            if p not in FP32_PAIRS:
                emit_conv(p)
            for s in range(2):
                if p in FP32_PAIRS:
                    pending.append((p, gram_job(p, s, 0)))
                    pending.append((p, gram_job(p, s, 1)))
                else:
                    pending.append((p, gram_job(p, s)))
            if tail:
                # tail-chain pairs: A copies join the pumped job stream;
                # G2 squarings for the last pumped pairs wait until after
                # the tail-pair prep (they are only needed by run2)
                pending.append((p, copy_job(p, 0)))
                pending.append((p, copy_job(p, 1)))
                if p < 3 * NPAIR // 4:
                    pending.append((p, sq_job(p, 0, 0)))
                    pending.append((p, sq_job(p, 0, 1)))

        def emit_copies(ps_list):
            for p in ps_list:
                copy_job(p, 0)()
                copy_job(p, 1)()

        pump_k = 4

        def emit_chain(g, run1="g2"):
            p0 = g * GP
            As = [Aall[:, p0 + j, :] for j in range(GP)]
            G2s = [G2all[:, p0 + j, :] for j in range(GP)]

            # ---- run1 matvec waves, fixups on Act (scalar engine).
            # Iterates stay unnormalized (power iteration is scale
            # invariant); the x9 fixup folds in 2^-52 so the Rayleigh
            # products stay in fp32 range.  Squared Grams shorten the
            # serial PSUM round-trip chain: x9 = G4 G4 G x0 (tail chain)
            # or G2^4 G x0 (chain 0, G4 copies not worth its window).
            if run1 == "g4":
                G4s = [G4all[:, j, :] for j in range(GP)]
                seq = [(As, "v"), (G4s, "v"), (G4s, "x9"), (As, "ray")]
            elif run1 == "g2":
                seq = [(As, "v")] + [(G2s, "v")] * 3 + [(G2s, "x9"), (As, "ray")]
            else:
                seq = [(As, "v")] * (NITER - 1) + [(As, "x9"), (As, "ray")]
            cur = X0[:, p0 : p0 + GP]
            x9s = None
            psw = None
            for i, (mats, kind) in enumerate(seq):
                psw = pspool.tile([PG, GP], F32, tag="mv", bufs=2, name=f"m1_{g}_{i}")
                for j in range(GP):
                    nc.tensor.matmul(
                        psw[:, j : j + 1], mats[j], cur[:, j : j + 1],
                        start=True, stop=True,
                    )
                pump(pump_k)
                if kind == "x9":
                    x9s = vpool.tile([PG, GP], F32, tag="x9", bufs=2, name=f"x9s{g}")
                    if g == NG - 1:
                        nc.vector.tensor_scalar(x9s[:], psw[:], S52, None, op0=ALU.mult)
                    else:
                        nc.scalar.mul(x9s[:], psw[:], S52)
                    cur = x9s[:]
                elif kind == "v":
                    nxt = vpool.tile([PG, GP], F32, tag="v", name=f"v1_{g}_{i}")
                    if g == NG - 1:
                        nc.vector.tensor_copy(nxt[:], psw[:])
                    else:
                        nc.scalar.copy(nxt[:], psw[:])
                    cur = nxt[:]

            # ---- Rayleigh 1 -> lambda in block layout (DVE) ------------
            # num/den = blockwise partition sums of x9*w and x9*x9 via the
            # block-diagonal-ones stationary: result lands broadcast on
            # every partition of the owning block.
            T = vpool.tile([PG, 2 * GP], F32, tag="T", bufs=2, name=f"T1_{g}")
            nc.vector.tensor_tensor(T[:, 0:GP], x9s[:], psw[:], op=ALU.mult)
            nc.vector.tensor_tensor(T[:, GP : 2 * GP], x9s[:], x9s[:], op=ALU.mult)
            pnd = pspool.tile([PG, 2 * GP], F32, tag="ray", bufs=1, name=f"pn1_{g}")
            nc.tensor.matmul(pnd[:], ONESB[:], T[:], start=True, stop=True)
            pump(pump_k)
            # +tiny while copying out of PSUM: on the junk partitions
            # outside the 49-blocks num = den = 0, and a bare reciprocal
            # would give lam = 0*inf = NaN there, which the fused run2
            # waves would then propagate into the blocks (0*NaN = NaN in
            # the matvec contraction).  With the bias those rows give
            # lam = tiny/tiny = 1.0, which multiplies the zero iterate
            # harmlessly.  On block rows den is ~1e29+, so the bias is
            # far below one ulp.
            ndc = vpool.tile([PG, 2 * GP], F32, tag="nd", bufs=2, name=f"nd1_{g}")
            nc.vector.tensor_scalar(ndc[:], pnd[:], 1e-30, None, op0=ALU.add)
            rd = vpool.tile([PG, GP], F32, tag="rd", bufs=2, name=f"rd{g}")
            nc.vector.reciprocal(rd[:], ndc[:, GP : 2 * GP])
            LAMV = vpool.tile([PG, GP], F32, tag="lam", bufs=2, name=f"lam{g}")
            nc.vector.tensor_tensor(LAMV[:], ndc[:, 0:GP], rd[:], op=ALU.mult)
            LAMV2 = vpool.tile([PG, GP], F32, tag="lam2", bufs=2, name=f"l2_{g}")
            nc.vector.tensor_scalar(LAMV2[:], LAMV[:], 2.0, None, op0=ALU.mult)
            LAMSQ = vpool.tile([PG, GP], F32, tag="lamsq", bufs=2, name=f"lq_{g}")
            nc.vector.tensor_tensor(LAMSQ[:], LAMV[:], LAMV[:], op=ALU.mult)
            x1 = vpool.tile([PG, GP], F32, tag="x1", bufs=2, name=f"x1_{g}")
            nc.scalar.mul(x1[:], x9s[:], S104)

            # ---- run2: fused shifted waves, B v = G v - lam*v.  The
            # lam*cur (and lam^2*cur) products are issued before the matvec
            # so they hide under it; fixups run on DVE.  The tail chain
            # applies B^2 = G2 - 2*lam*G + lam^2 per wave, halving the
            # number of serial PSUM round trips.
            cur = x1[:]
            u9 = None
            steps = ["s"] + ["d"] * 4 + ["s"]
            u9_at = 4
            for i, kind in enumerate(steps):
                if kind == "s":
                    t = vpool.tile([PG, GP], F32, tag="v", name=f"t2_{g}_{i}")
                    nc.vector.tensor_tensor(t[:], LAMV[:], cur, op=ALU.mult)
                    psw = pspool.tile(
                        [PG, GP], F32, tag="mv", bufs=2, name=f"m2_{g}_{i}"
                    )
                    for j in range(GP):
                        nc.tensor.matmul(
                            psw[:, j : j + 1], As[j], cur[:, j : j + 1],
                            start=True, stop=True,
                        )
                    pump(pump_k)
                    nxt = vpool.tile([PG, GP], F32, tag="v", name=f"v2_{g}_{i}")
                    nc.vector.tensor_tensor(nxt[:], psw[:], t[:], op=ALU.subtract)
                else:
                    e = vpool.tile([PG, GP], F32, tag="v", name=f"e2_{g}_{i}")
                    nc.vector.tensor_tensor(e[:], LAMSQ[:], cur, op=ALU.mult)
                    psa = pspool.tile(
                        [PG, GP], F32, tag="mv", bufs=2, name=f"m2a_{g}_{i}"
                    )
                    psb = pspool.tile(
                        [PG, GP], F32, tag="mv", bufs=2, name=f"m2b_{g}_{i}"
                    )
                    for j in range(GP):
                        nc.tensor.matmul(
                            psa[:, j : j + 1], G2s[j], cur[:, j : j + 1],
                            start=True, stop=True,
                        )
                    for j in range(GP):
                        nc.tensor.matmul(
                            psb[:, j : j + 1], As[j], cur[:, j : j + 1],
                            start=True, stop=True,
                        )
                    # the two PSUM reads are independent so their access
                    # latencies overlap; only the final sbuf-sbuf subtract
                    # waits on both
                    t1 = vpool.tile([PG, GP], F32, tag="v", name=f"t1_{g}_{i}")
                    nc.vector.tensor_tensor(t1[:], psa[:], e[:], op=ALU.add)
                    d1 = vpool.tile([PG, GP], F32, tag="v", name=f"d1_{g}_{i}")
                    nc.vector.tensor_tensor(d1[:], LAMV2[:], psb[:], op=ALU.mult)
                    nxt = vpool.tile([PG, GP], F32, tag="v", name=f"v2_{g}_{i}")
                    nc.vector.tensor_tensor(nxt[:], t1[:], d1[:], op=ALU.subtract)
                if i == u9_at:
                    u9 = nxt
                cur = nxt[:]
            w2 = cur
            # remaining queued tail-pair prep: emitted here (PE idle gaps
            # between this chain's end and the tail chain) rather than
            # behind this chain's R2 matmul
            flush()

            # ---- Rayleigh 2 + penalty: pen = (n2 / (n2 + lam*d2))^2 ----
            T2 = vpool.tile([PG, 2 * GP], F32, tag="T", bufs=2, name=f"T2_{g}")
            nc.vector.tensor_tensor(T2[:, 0:GP], u9[:], w2, op=ALU.mult)
            nc.vector.tensor_tensor(T2[:, GP : 2 * GP], u9[:], u9[:], op=ALU.mult)
            pnd2 = pspool.tile([PG, 2 * GP], F32, tag="ray", bufs=1, name=f"pn2_{g}")
            nc.tensor.matmul(pnd2[:], ONESB[:], T2[:], start=True, stop=True)
            nd2 = vpool.tile([PG, 2 * GP], F32, tag="nd", bufs=2, name=f"nd2_{g}")
            nc.vector.tensor_copy(nd2[:], pnd2[:])
            t2v = vpool.tile([PG, GP], F32, tag="rd", bufs=2, name=f"t2v{g}")
            nc.vector.tensor_tensor(t2v[:], LAMV[:], nd2[:, GP : 2 * GP], op=ALU.mult)
            qv = vpool.tile([PG, GP], F32, tag="q", bufs=2, name=f"qv{g}")
            nc.vector.tensor_tensor(qv[:], nd2[:, 0:GP], t2v[:], op=ALU.add)
            rq = vpool.tile([PG, GP], F32, tag="rq", bufs=2, name=f"rq{g}")
            nc.vector.reciprocal(rq[:], qv[:])
            rt = vpool.tile([PG, GP], F32, tag="rt", bufs=2, name=f"rt{g}")
            nc.vector.tensor_tensor(rt[:], nd2[:, 0:GP], rq[:], op=ALU.mult)
            PENg = vpool.tile([PG, GP], F32, tag="pen", bufs=2, name=f"pen{g}")
            nc.vector.tensor_tensor(PENg[:], rt[:], rt[:], op=ALU.mult)

            # pen[8g + 2j + e] = PENg[64e, j]; deferred to SP after all x
            # DMAs so no engine's stream queues behind a chain-gated DMA.
            pen_r = pen.rearrange("(g j e) -> g e j", g=NG, e=2)
            PEN_v = PENg.rearrange("(b q) p -> b q p", b=2)[:, 0, :]
            pen_dmas.append((pen_r[g], PEN_v))

        groups = [list(range(g * GP, (g + 1) * GP)) for g in range(NG)]

        TAIL = (NPAIR - 2, NPAIR - 1)
        for p in groups[0]:
            queue_pair(p)
        flush()
        emit_copies(groups[0])
        for p in groups[1]:
            if p in TAIL:
                emit_dma(p, nsub=2 if p == NPAIR - 1 else 1)
            else:
                queue_pair(p, tail=True)
        for p in groups[0]:
            sq_job(p, 0, 0)()
            sq_job(p, 0, 1)()
        emit_chain(0, run1="g2")
        flush()
        # tail-pair prep lands in the post-chain-0 window where PE, DVE
        # and Act are all otherwise idle
        # tail prep is the serial gate into chain 1: split every step
        # across DVE and Act so the two engines halve it between them
        for p in TAIL:
            emit_conv(p)
            if p == NPAIR - 1:
                gram_job(p, 0)()
                gram_job(p, 1, 0)()
                gram_job(p, 1, 1)()
            else:
                gram_job(p, 0)()
                gram_job(p, 1)()
        # all four A copies ahead of any squaring (wave 1 only needs A;
        # G2 is not used until run2), block-a on DVE, block-b on Act
        for p in TAIL:
            pg_ = pair_ps.pop(p)
            nc.vector.tensor_copy(Aall[0:N, p, 0:N], pg_[0:N, :])
            nc.vector.tensor_copy(Aall[B1 : B1 + N, p, B1 : B1 + N], pg_[B1 : B1 + N, :])
        for p in TAIL:
            sq_job(p, 0, 0)()
            sq_job(p, 0, 1)()
        for p in range(3 * NPAIR // 4, NPAIR - 2):
            sq_job(p, 0, 0)()
            sq_job(p, 0, 1)()
        emit_chain(1, run1="g2")
        for dst, src in pen_dmas:
            nc.sync.dma_start(dst, src)


_NC_CACHE = {}


def build_nc(repeat=1):
    if repeat in _NC_CACHE:
        return _NC_CACHE[repeat]
    nc = bacc.Bacc("TRN2", target_bir_lowering=False, debug=False)
    x = nc.dram_tensor("x", [BS, C, N], F32, kind="ExternalInput")
    x0 = nc.dram_tensor("x0", [BS, N], F32, kind="ExternalInput")
    pen = nc.dram_tensor("pen", [BS], F32, kind="ExternalOutput")
    with tile.TileContext(nc) as tc:
        _emit(tc, x.ap(), x0.ap(), pen.ap())
    nc.compile()
    _NC_CACHE[repeat] = nc
    return nc


LAST_RESULTS = None


def kernel(x, x0):
    global LAST_RESULTS
    x = np.ascontiguousarray(np.asarray(x, dtype=np.float32).reshape(B, C, N))
    x0 = np.ascontiguousarray(np.asarray(x0, dtype=np.float32).reshape(B, N))
    nc = build_nc()
    in_maps = [
        {"x": x[i * BS : (i + 1) * BS], "x0": x0[i * BS : (i + 1) * BS]}
        for i in range(NCORES)
    ]
    trace = bool(int(os.environ.get("KERNEL_TRACE", "0")))
    res = run_bass_kernel_spmd(nc, in_maps, list(range(NCORES)), trace=trace)
    LAST_RESULTS = res
    pens = np.concatenate([r["pen"].reshape(-1) for r in res.results])
    return np.float32(pens.sum(dtype=np.float64) / B)


# revision 70
# speedup vs baseline: 1.3282x; 1.0016x over previous
"""Trainium2 Bass kernel for the OFPenalty eigenvalue-penalty loss.

Math (per sample b of 256):
  W = x[b] reshaped [C=2048, N=49];  G = W^T W  (49x49 Gram matrix)
  run1: x9 = G^9 x0 (power iteration, normalization deferred - scale
        invariant), largest = Rayleigh(G, x9) = x9^T G x9 / x9^T x9
  run2: B = G - largest*I applied fused per wave (never materialized),
        u9 = B^9 x1 (x1 = scaled x9), tmp = Rayleigh(B, u9)
  penalty = (tmp/(tmp+largest))^2 ; output = mean over batch.

Layout: pure data parallel, 32 samples per core on 8 cores.  Samples
are packed in pairs block-diagonally: sample 2p on partitions 0:49,
sample 2p+1 on 64:113.  The Gram inputs are converted fp32->fp16 so
the 512 Gram matmuls run at 1 cycle/row instead of 4 (PSUM still
accumulates fp32; rel err stays ~1e-4, far under the 2e-2 gate).

Pipelining: the 16 pairs stream in DMA order in two groups of 8.
Group 0's eigen-chain (run1, Rayleigh/lambda, fused run2, penalty)
runs while group 1's DMA + conversion + Grams continue; group 1's prep
jobs are pumped into the engine gaps between chain-0 waves, and only
the tail pair's prep plus chain 1 trail the final DMA.  Serial PSUM
round trips are halved with squared Grams: run1 applies G2 (x9 =
G2^4 G x0, 6 waves) and run2 applies B^2 = G2 - 2*lam*G + lam^2
per double wave (B^9 = B^2^4 B, 6 waves).

Rayleigh sums use a block-diagonal-ones stationary so the per-sample
numerator/denominator land broadcast across all partitions in block
layout - lambda feeds the fused run2 with no rank-1 rebroadcast.
Iterate overflow is handled by folding exact powers-of-two scales into
single fixup/scale ops (wave 9 of run1, and the run2 warm start).
"""

import os
import sys
from contextlib import ExitStack

import numpy as np

for _p in ("/opt/trn_rl_repo",):
    if os.path.isdir(_p) and _p not in sys.path:
        sys.path.insert(0, _p)

import concourse.bass as bass  # noqa: E402
import concourse.tile as tile  # noqa: E402
from concourse import bacc, mybir  # noqa: E402
from concourse.bass_utils import run_bass_kernel_spmd  # noqa: E402

F32 = mybir.dt.float32
F16 = mybir.dt.float16
ALU = mybir.AluOpType

B, C, N = 256, 2048, 49
NCORES = 8
BS = B // NCORES  # 32 samples per core
NPAIR = BS // 2  # 16 pairs
KT = C // 128  # 16 contraction tiles
PG = 128  # gapped pair-vector space: blocks at [0:49], [64:113]
B1 = 64  # partition base of the second sample in a pair
NG = 2  # pipeline groups
GP = NPAIR // NG  # pairs per group
NITER = 9
S52 = float(2.0**-52)  # rescale x9 before Rayleigh products
S104 = float(2.0**-104)  # rescale x9 -> x1 (run2 warm start)


def _emit(tc, x, x0, pen):
    nc = tc.nc
    ctx = ExitStack()
    with ctx:
        const = ctx.enter_context(tc.tile_pool(name="const", bufs=1))
        xpool = ctx.enter_context(tc.tile_pool(name="xt", bufs=8))
        hpool = ctx.enter_context(tc.tile_pool(name="xh", bufs=16))
        vpool = ctx.enter_context(tc.tile_pool(name="vec", bufs=6))
        pspool = ctx.enter_context(tc.tile_pool(name="ps", bufs=1, space="PSUM"))

        # ---- constants -------------------------------------------------
        # block-diagonal ones: partition-sum broadcast within each block
        ONESB = const.tile([PG, PG], F32)
        nc.gpsimd.memset(ONESB[:], 0.0)
        nc.gpsimd.memset(ONESB[0:N, 0:N], 1.0)
        nc.gpsimd.memset(ONESB[B1 : B1 + N, B1 : B1 + N], 1.0)
        # x0 columns: X0[0:49, p] = x0[2p], X0[64:113, p] = x0[2p+1].
        # Loaded via a contiguous [32, 49] DMA + PE transpose: a direct
        # strided DMA would burn ~0.7us of 4-byte descriptors on the DMA
        # engines ahead of the x stream.
        X0 = const.tile([PG, NPAIR], F32)
        nc.gpsimd.memset(X0[:], 0.0)
        IDT = const.tile([32, 32], F32)
        nc.gpsimd.memset(IDT[:], 0.0)
        nc.gpsimd.affine_select(
            out=IDT[:], in_=IDT[:], compare_op=ALU.not_equal, fill=1.0,
            base=0, pattern=[[-1, 32]], channel_multiplier=1,
        )
        S0 = const.tile([32, N], F32)
        # block-diagonal Gram matrices (off-block stays zero)
        Aall = const.tile([PG, NPAIR, PG], F32)
        nc.gpsimd.memset(Aall[:], 0.0)
        # squared Grams: G2 drives the B^2 double waves of run2 for every
        # chain and the 6-wave run1 of chain 0; G4 (tail pairs only) gives
        # chain 1 a 4-wave run1 (x9 = G4 G4 G x0)
        G2all = const.tile([PG, NPAIR, PG], F32)
        G4all = const.tile([PG, GP, PG], F32)

        # per-sample DMA view: partition q holds c-rows {512b + 4q + r},
        # 784B-contiguous descriptors (full DMA bandwidth); the (b, r)
        # enumeration of contraction tiles is a permutation of c, which
        # the Gram sum is invariant to.
        xrs = x.rearrange("s (b q r) j -> s q b (r j)", b=4, q=128, r=4)

        # DVE converts at 0.5 cyc/elem (2x mode), Act at 1/1.2GHz, Pool at
        # 1/(1.2GHz*0.6).  DVE takes the early evens (it is chain-free until
        # ~22us), Act the early odds plus the tail pair (it is free when the
        # tail arrives), Pool the middle stretch.
        # tail pairs (14-15) convert on DVE/Act right after chain 0's
        # engine blocks clear - their conv+gram prep is the gate into the
        # tail chain, so it gets the fastest converters.
        FP32_PAIRS = set()
        _CONV = {}
        for _idx in range(2 * NPAIR):
            if _idx <= 15:
                _CONV[_idx] = "dve" if _idx % 2 == 0 else "act"
            elif _idx >= 28:
                _CONV[_idx] = {28: "dve", 29: "pool", 30: "dve", 31: "pool"}[_idx]
            else:
                _CONV[_idx] = "pool"

        def conv_eng(idx):
            return {"dve": nc.vector, "act": nc.scalar, "pool": nc.gpsimd}[
                _CONV[idx]
            ]

        xt_tiles = {}
        xh_tiles = {}
        pair_ps = {}
        pending = []  # deferred half-sample Gram emitters, pumped into chains
        pen_dmas = []

        def emit_dma(p, nsub=1):
            for s in range(2):
                idx = 2 * p + s
                xt = xpool.tile([PG, KT * N], F32, tag="xt", bufs=8, name=f"xt{idx}")
                xv = xt.rearrange("q (b m) -> q b m", b=4)
                for u in range(nsub):
                    lo, hi = u * 4 // nsub, (u + 1) * 4 // nsub
                    nc.sync.dma_start(xv[:, lo:hi], xrs[idx, :, lo:hi])
                xt_tiles[idx] = xt
                if idx == 0:
                    nc.sync.dma_start(S0[:], x0)
                    pst = pspool.tile([N, 32], F32, tag="ray", bufs=1, name="pst")
                    nc.tensor.transpose(pst[:], S0[:], IDT[:])
                    pst_r = pst.rearrange("j (p e) -> e j p", e=2)
                    nc.vector.tensor_copy(X0[0:N, :], pst_r[0])
                    nc.vector.tensor_copy(X0[B1 : B1 + N, :], pst_r[1])

        def _copy(eng, out, in_):
            if eng is nc.scalar:
                eng.copy(out, in_)
            else:
                eng.tensor_copy(out, in_)

        def emit_conv(p):
            halves = 2 if p >= NPAIR - 2 else 1
            for s in range(2):
                idx = 2 * p + s
                xh = hpool.tile([PG, KT * N], F16, tag="xh", name=f"xh{idx}")
                half = KT * N // 2
                for u in range(halves):
                    lo, hi = (u * half, (u + 1) * half) if halves == 2 else (0, KT * N)
                    _copy(conv_eng(idx), xh[:, lo:hi], xt_tiles[idx][:, lo:hi])
                xh_tiles[idx] = xh

        def gram_job(p, s, h=None):
            # h=None: the whole sample (16 tiles, ~0.33us fp16); h=0/1: one
            # half - used for the fp32 tail pairs so their (4x costlier)
            # matmuls pipeline with the per-half sub-DMAs
            def job():
                if s == 0 and h in (None, 0):
                    pair_ps[p] = pspool.tile(
                        [PG, N], F32, tag="gram", bufs=3, name=f"pg{p}"
                    )
                pg_ = pair_ps[p]
                src = xt_tiles[2 * p + s] if p in FP32_PAIRS else xh_tiles[2 * p + s]
                ob = 0 if s == 0 else B1
                ks = range(KT) if h is None else range(8 * h, 8 * h + 8)
                for k in ks:
                    wk = src[:, k * N : (k + 1) * N]
                    nc.tensor.matmul(
                        pg_[ob : ob + N, :], wk, wk,
                        start=(k == 0), stop=(k == KT - 1),
                    )
            return job

        def copy_job(p, s):
            def job():
                pg_ = pair_ps[p]
                if s == 0:
                    nc.scalar.copy(Aall[0:N, p, 0:N], pg_[0:N, :])
                else:
                    nc.scalar.copy(
                        Aall[B1 : B1 + N, p, B1 : B1 + N], pg_[B1 : B1 + N, :]
                    )
            return job

        def sq_job(p, stage, part):
            # stage 0: G2 = A*A, stage 1: G4 = G2*G2; part 0 = PE matmul,
            # part 1 = PSUM->SBUF copy (DVE mid-stream; Act for the tail
            # pairs, since DVE is still busy with chain 0 when they land)
            def job():
                src = Aall[:, p, :] if stage == 0 else G2all[:, p, :]
                dst = G2all[:, p, :] if stage == 0 else G4all[:, p - GP, :]
                if part == 0:
                    ps2 = pspool.tile(
                        [PG, PG], F32, tag="sq", bufs=2, name=f"sq{p}_{stage}"
                    )
                    pair_ps[("sq", p)] = ps2
                    nc.tensor.matmul(ps2[:], src, src, start=True, stop=True)
                elif p >= NPAIR - 2:
                    nc.scalar.copy(dst, pair_ps.pop(("sq", p))[:])
                else:
                    nc.vector.tensor_copy(dst, pair_ps.pop(("sq", p))[:])
            return job

        # Jobs are (pair, thunk).  pump() only feeds jobs whose pair's data
        # lands while chain 0 is running (pairs < PUMP_CUTOFF); later pairs
        # would stall the chain waves on their DMA, so they run in the
        # post-chain-0 engine-idle window instead (via flush).
        PUMP_CUTOFF = NPAIR - 2

        def pump(k=1):
            for _ in range(k):
                if pending and pending[0][0] < PUMP_CUTOFF:
                    pending.pop(0)[1]()

        def flush():
            while pending:
                pending.pop(0)[1]()

        def queue_pair(p, tail=False):
            emit_dma(p, nsub=2 if p in FP32_PAIRS else 1)
            if p not in FP32_PAIRS:
                emit_conv(p)
            for s in range(2):
                if p in FP32_PAIRS:
                    pending.append((p, gram_job(p, s, 0)))
                    pending.append((p, gram_job(p, s, 1)))
                else:
                    pending.append((p, gram_job(p, s)))
            if tail:
                # tail-chain pairs: A copies join the pumped job stream;
                # G2 squarings for the last pumped pairs wait until after
                # the tail-pair prep (they are only needed by run2)
                pending.append((p, copy_job(p, 0)))
                pending.append((p, copy_job(p, 1)))
                if p < 3 * NPAIR // 4:
                    pending.append((p, sq_job(p, 0, 0)))
                    pending.append((p, sq_job(p, 0, 1)))

        def emit_copies(ps_list):
            for p in ps_list:
                copy_job(p, 0)()
                copy_job(p, 1)()

        pump_k = 4

        def emit_chain(g, run1="g2"):
            p0 = g * GP
            As = [Aall[:, p0 + j, :] for j in range(GP)]
            G2s = [G2all[:, p0 + j, :] for j in range(GP)]

            # ---- run1 matvec waves, fixups on Act (scalar engine).
            # Iterates stay unnormalized (power iteration is scale
            # invariant); the x9 fixup folds in 2^-52 so the Rayleigh
            # products stay in fp32 range.  Squared Grams shorten the
            # serial PSUM round-trip chain: x9 = G4 G4 G x0 (tail chain)
            # or G2^4 G x0 (chain 0, G4 copies not worth its window).
            if run1 == "g4":
                G4s = [G4all[:, j, :] for j in range(GP)]
                seq = [(As, "v"), (G4s, "v"), (G4s, "x9"), (As, "ray")]
            elif run1 == "g2":
                seq = [(As, "v")] + [(G2s, "v")] * 3 + [(G2s, "x9"), (As, "ray")]
            else:
                seq = [(As, "v")] * (NITER - 1) + [(As, "x9"), (As, "ray")]
            cur = X0[:, p0 : p0 + GP]
            x9s = None
            psw = None
            for i, (mats, kind) in enumerate(seq):
                psw = pspool.tile([PG, GP], F32, tag="mv", bufs=2, name=f"m1_{g}_{i}")
                for j in range(GP):
                    nc.tensor.matmul(
                        psw[:, j : j + 1], mats[j], cur[:, j : j + 1],
                        start=True, stop=True,
                    )
                pump(pump_k)
                if kind == "x9":
                    x9s = vpool.tile([PG, GP], F32, tag="x9", bufs=2, name=f"x9s{g}")
                    if g == NG - 1:
                        nc.vector.tensor_scalar(x9s[:], psw[:], S52, None, op0=ALU.mult)
                    else:
                        nc.scalar.mul(x9s[:], psw[:], S52)
                    cur = x9s[:]
                elif kind == "v":
                    nxt = vpool.tile([PG, GP], F32, tag="v", name=f"v1_{g}_{i}")
                    if g == NG - 1:
                        nc.vector.tensor_copy(nxt[:], psw[:])
                    else:
                        nc.scalar.copy(nxt[:], psw[:])
                    cur = nxt[:]

            # ---- Rayleigh 1 -> lambda in block layout (DVE) ------------
            # num/den = blockwise partition sums of x9*w and x9*x9 via the
            # block-diagonal-ones stationary: result lands broadcast on
            # every partition of the owning block.
            T = vpool.tile([PG, 2 * GP], F32, tag="T", bufs=2, name=f"T1_{g}")
            nc.vector.tensor_tensor(T[:, 0:GP], x9s[:], psw[:], op=ALU.mult)
            nc.vector.tensor_tensor(T[:, GP : 2 * GP], x9s[:], x9s[:], op=ALU.mult)
            pnd = pspool.tile([PG, 2 * GP], F32, tag="ray", bufs=1, name=f"pn1_{g}")
            nc.tensor.matmul(pnd[:], ONESB[:], T[:], start=True, stop=True)
            pump(pump_k)
            # +tiny while copying out of PSUM: on the junk partitions
            # outside the 49-blocks num = den = 0, and a bare reciprocal
            # would give lam = 0*inf = NaN there, which the fused run2
            # waves would then propagate into the blocks (0*NaN = NaN in
            # the matvec contraction).  With the bias those rows give
            # lam = tiny/tiny = 1.0, which multiplies the zero iterate
            # harmlessly.  On block rows den is ~1e29+, so the bias is
            # far below one ulp.
            ndc = vpool.tile([PG, 2 * GP], F32, tag="nd", bufs=2, name=f"nd1_{g}")
            nc.vector.tensor_scalar(ndc[:], pnd[:], 1e-30, None, op0=ALU.add)
            rd = vpool.tile([PG, GP], F32, tag="rd", bufs=2, name=f"rd{g}")
            nc.vector.reciprocal(rd[:], ndc[:, GP : 2 * GP])
            LAMV = vpool.tile([PG, GP], F32, tag="lam", bufs=2, name=f"lam{g}")
            nc.vector.tensor_tensor(LAMV[:], ndc[:, 0:GP], rd[:], op=ALU.mult)
            LAMV2 = vpool.tile([PG, GP], F32, tag="lam2", bufs=2, name=f"l2_{g}")
            nc.vector.tensor_scalar(LAMV2[:], LAMV[:], 2.0, None, op0=ALU.mult)
            LAMSQ = vpool.tile([PG, GP], F32, tag="lamsq", bufs=2, name=f"lq_{g}")
            nc.vector.tensor_tensor(LAMSQ[:], LAMV[:], LAMV[:], op=ALU.mult)
            x1 = vpool.tile([PG, GP], F32, tag="x1", bufs=2, name=f"x1_{g}")
            nc.scalar.mul(x1[:], x9s[:], S104)

            # ---- run2: fused shifted waves, B v = G v - lam*v.  The
            # lam*cur (and lam^2*cur) products are issued before the matvec
            # so they hide under it; fixups run on DVE.  The tail chain
            # applies B^2 = G2 - 2*lam*G + lam^2 per wave, halving the
            # number of serial PSUM round trips.
            cur = x1[:]
            u9 = None
            steps = ["s"] + ["d"] * 4 + ["s"]
            u9_at = 4
            for i, kind in enumerate(steps):
                if kind == "s":
                    t = vpool.tile([PG, GP], F32, tag="v", name=f"t2_{g}_{i}")
                    nc.vector.tensor_tensor(t[:], LAMV[:], cur, op=ALU.mult)
                    psw = pspool.tile(
                        [PG, GP], F32, tag="mv", bufs=2, name=f"m2_{g}_{i}"
                    )
                    for j in range(GP):
                        nc.tensor.matmul(
                            psw[:, j : j + 1], As[j], cur[:, j : j + 1],
                            start=True, stop=True,
                        )
                    pump(pump_k)
                    nxt = vpool.tile([PG, GP], F32, tag="v", name=f"v2_{g}_{i}")
                    nc.vector.tensor_tensor(nxt[:], psw[:], t[:], op=ALU.subtract)
                else:
                    e = vpool.tile([PG, GP], F32, tag="v", name=f"e2_{g}_{i}")
                    nc.vector.tensor_tensor(e[:], LAMSQ[:], cur, op=ALU.mult)
                    psa = pspool.tile(
                        [PG, GP], F32, tag="mv", bufs=2, name=f"m2a_{g}_{i}"
                    )
                    psb = pspool.tile(
                        [PG, GP], F32, tag="mv", bufs=2, name=f"m2b_{g}_{i}"
                    )
                    for j in range(GP):
                        nc.tensor.matmul(
                            psa[:, j : j + 1], G2s[j], cur[:, j : j + 1],
                            start=True, stop=True,
                        )
                    for j in range(GP):
                        nc.tensor.matmul(
                            psb[:, j : j + 1], As[j], cur[:, j : j + 1],
                            start=True, stop=True,
                        )
                    # the two PSUM reads are independent so their access
                    # latencies overlap; only the final sbuf-sbuf subtract
                    # waits on both
                    t1 = vpool.tile([PG, GP], F32, tag="v", name=f"t1_{g}_{i}")
                    nc.vector.tensor_tensor(t1[:], psa[:], e[:], op=ALU.add)
                    d1 = vpool.tile([PG, GP], F32, tag="v", name=f"d1_{g}_{i}")
                    nc.vector.tensor_tensor(d1[:], LAMV2[:], psb[:], op=ALU.mult)
                    nxt = vpool.tile([PG, GP], F32, tag="v", name=f"v2_{g}_{i}")
                    nc.vector.tensor_tensor(nxt[:], t1[:], d1[:], op=ALU.subtract)
                if i == u9_at:
                    u9 = nxt
                cur = nxt[:]
            w2 = cur
            # remaining queued tail-pair prep: emitted here (PE idle gaps
            # between this chain's end and the tail chain) rather than
            # behind this chain's R2 matmul
            flush()

            # ---- Rayleigh 2 + penalty: pen = (n2 / (n2 + lam*d2))^2 ----
            T2 = vpool.tile([PG, 2 * GP], F32, tag="T", bufs=2, name=f"T2_{g}")
            nc.vector.tensor_tensor(T2[:, 0:GP], u9[:], w2, op=ALU.mult)
            nc.vector.tensor_tensor(T2[:, GP : 2 * GP], u9[:], u9[:], op=ALU.mult)
            pnd2 = pspool.tile([PG, 2 * GP], F32, tag="ray", bufs=1, name=f"pn2_{g}")
            nc.tensor.matmul(pnd2[:], ONESB[:], T2[:], start=True, stop=True)
            nd2 = vpool.tile([PG, 2 * GP], F32, tag="nd", bufs=2, name=f"nd2_{g}")
            nc.vector.tensor_copy(nd2[:], pnd2[:])
            t2v = vpool.tile([PG, GP], F32, tag="rd", bufs=2, name=f"t2v{g}")
            nc.vector.tensor_tensor(t2v[:], LAMV[:], nd2[:, GP : 2 * GP], op=ALU.mult)
            qv = vpool.tile([PG, GP], F32, tag="q", bufs=2, name=f"qv{g}")
            nc.vector.tensor_tensor(qv[:], nd2[:, 0:GP], t2v[:], op=ALU.add)
            rq = vpool.tile([PG, GP], F32, tag="rq", bufs=2, name=f"rq{g}")
            nc.vector.reciprocal(rq[:], qv[:])
            rt = vpool.tile([PG, GP], F32, tag="rt", bufs=2, name=f"rt{g}")
            nc.vector.tensor_tensor(rt[:], nd2[:, 0:GP], rq[:], op=ALU.mult)
            PENg = vpool.tile([PG, GP], F32, tag="pen", bufs=2, name=f"pen{g}")
            nc.vector.tensor_tensor(PENg[:], rt[:], rt[:], op=ALU.mult)

            # pen[8g + 2j + e] = PENg[64e, j]; deferred to SP after all x
            # DMAs so no engine's stream queues behind a chain-gated DMA.
            pen_r = pen.rearrange("(g j e) -> g e j", g=NG, e=2)
            PEN_v = PENg.rearrange("(b q) p -> b q p", b=2)[:, 0, :]
            pen_dmas.append((pen_r[g], PEN_v))

        groups = [list(range(g * GP, (g + 1) * GP)) for g in range(NG)]

        TAIL = (NPAIR - 2, NPAIR - 1)
        for p in groups[0]:
            queue_pair(p)
        flush()
        emit_copies(groups[0])
        for p in groups[1]:
            if p in TAIL:
                emit_dma(p, nsub=2 if p == NPAIR - 1 else 1)
            else:
                queue_pair(p, tail=True)
        for p in groups[0]:
            sq_job(p, 0, 0)()
            sq_job(p, 0, 1)()
        emit_chain(0, run1="g2")
        flush()
        # tail-pair prep lands in the post-chain-0 window where PE, DVE
        # and Act are all otherwise idle
        # tail prep is the serial gate into chain 1: split every step
        # across DVE and Act so the two engines halve it between them
        for p in TAIL:
            emit_conv(p)
            if p == NPAIR - 1:
                gram_job(p, 0)()
                gram_job(p, 1, 0)()
                gram_job(p, 1, 1)()
            else:
                gram_job(p, 0)()
                gram_job(p, 1)()
        # all four A copies ahead of any squaring (wave 1 only needs A;
        # G2 is not used until run2), block-a on DVE, block-b on Act
        for p in TAIL:
            pg_ = pair_ps.pop(p)
            nc.vector.tensor_copy(Aall[0:N, p, 0:N], pg_[0:N, :])
            nc.vector.tensor_copy(Aall[B1 : B1 + N, p, B1 : B1 + N], pg_[B1 : B1 + N, :])
        for p in TAIL:
            sq_job(p, 0, 0)()
            sq_job(p, 0, 1)()
        for p in range(3 * NPAIR // 4, NPAIR - 2):
            sq_job(p, 0, 0)()
            sq_job(p, 0, 1)()
        emit_chain(1, run1="g2")
        for dst, src in pen_dmas:
            nc.sync.dma_start(dst, src)


_NC_CACHE = {}


def build_nc(repeat=1):
    if repeat in _NC_CACHE:
        return _NC_CACHE[repeat]
    nc = bacc.Bacc("TRN2", target_bir_lowering=False, debug=False)
    x = nc.dram_tensor("x", [BS, C, N], F32, kind="ExternalInput")
    x0 = nc.dram_tensor("x0", [BS, N], F32, kind="ExternalInput")
    pen = nc.dram_tensor("pen", [BS], F32, kind="ExternalOutput")
    with tile.TileContext(nc) as tc:
        _emit(tc, x.ap(), x0.ap(), pen.ap())
    nc.compile()
    _NC_CACHE[repeat] = nc
    return nc


LAST_RESULTS = None


def kernel(x, x0):
    global LAST_RESULTS
    x = np.ascontiguousarray(np.asarray(x, dtype=np.float32).reshape(B, C, N))
    x0 = np.ascontiguousarray(np.asarray(x0, dtype=np.float32).reshape(B, N))
    nc = build_nc()
    in_maps = [
        {"x": x[i * BS : (i + 1) * BS], "x0": x0[i * BS : (i + 1) * BS]}
        for i in range(NCORES)
    ]
    trace = bool(int(os.environ.get("KERNEL_TRACE", "0")))
    res = run_bass_kernel_spmd(nc, in_maps, list(range(NCORES)), trace=trace)
    LAST_RESULTS = res
    pens = np.concatenate([r["pen"].reshape(-1) for r in res.results])
    return np.float32(pens.sum(dtype=np.float64) / B)


# revision 71
# speedup vs baseline: 1.3343x; 1.0046x over previous
"""Trainium2 Bass kernel for the OFPenalty eigenvalue-penalty loss.

Math (per sample b of 256):
  W = x[b] reshaped [C=2048, N=49];  G = W^T W  (49x49 Gram matrix)
  run1: x9 = G^9 x0 (power iteration, normalization deferred - scale
        invariant), largest = Rayleigh(G, x9) = x9^T G x9 / x9^T x9
  run2: B = G - largest*I applied fused per wave (never materialized),
        u9 = B^9 x1 (x1 = scaled x9), tmp = Rayleigh(B, u9)
  penalty = (tmp/(tmp+largest))^2 ; output = mean over batch.

Layout: pure data parallel, 32 samples per core on 8 cores.  Samples
are packed in pairs block-diagonally: sample 2p on partitions 0:49,
sample 2p+1 on 64:113.  The Gram inputs are converted fp32->fp16 so
the 512 Gram matmuls run at 1 cycle/row instead of 4 (PSUM still
accumulates fp32; rel err stays ~1e-4, far under the 2e-2 gate).

Pipelining: the 16 pairs stream in DMA order in two groups of 8.
Group 0's eigen-chain (run1, Rayleigh/lambda, fused run2, penalty)
runs while group 1's DMA + conversion + Grams continue; group 1's prep
jobs are pumped into the engine gaps between chain-0 waves, and only
the tail pair's prep plus chain 1 trail the final DMA.  Serial PSUM
round trips are halved with squared Grams: run1 applies G2 (x9 =
G2^4 G x0, 6 waves) and run2 applies B^2 = G2 - 2*lam*G + lam^2
per double wave (B^9 = B^2^4 B, 6 waves).

Rayleigh sums use a block-diagonal-ones stationary so the per-sample
numerator/denominator land broadcast across all partitions in block
layout - lambda feeds the fused run2 with no rank-1 rebroadcast.
Iterate overflow is handled by folding exact powers-of-two scales into
single fixup/scale ops (wave 9 of run1, and the run2 warm start).
"""

import os
import sys
from contextlib import ExitStack

import numpy as np

for _p in ("/opt/trn_rl_repo",):
    if os.path.isdir(_p) and _p not in sys.path:
        sys.path.insert(0, _p)

import concourse.bass as bass  # noqa: E402
import concourse.tile as tile  # noqa: E402
from concourse import bacc, mybir  # noqa: E402
from concourse.bass_utils import run_bass_kernel_spmd  # noqa: E402

F32 = mybir.dt.float32
F16 = mybir.dt.float16
ALU = mybir.AluOpType

B, C, N = 256, 2048, 49
NCORES = 8
BS = B // NCORES  # 32 samples per core
NPAIR = BS // 2  # 16 pairs
KT = C // 128  # 16 contraction tiles
PG = 128  # gapped pair-vector space: blocks at [0:49], [64:113]
B1 = 64  # partition base of the second sample in a pair
NG = 2  # pipeline groups
GP = NPAIR // NG  # pairs per group
NITER = 9
S52 = float(2.0**-52)  # rescale x9 before Rayleigh products
S104 = float(2.0**-104)  # rescale x9 -> x1 (run2 warm start)


def _emit(tc, x, x0, pen):
    nc = tc.nc
    ctx = ExitStack()
    with ctx:
        const = ctx.enter_context(tc.tile_pool(name="const", bufs=1))
        xpool = ctx.enter_context(tc.tile_pool(name="xt", bufs=8))
        hpool = ctx.enter_context(tc.tile_pool(name="xh", bufs=16))
        vpool = ctx.enter_context(tc.tile_pool(name="vec", bufs=6))
        pspool = ctx.enter_context(tc.tile_pool(name="ps", bufs=1, space="PSUM"))

        # ---- constants -------------------------------------------------
        # block-diagonal ones: partition-sum broadcast within each block
        ONESB = const.tile([PG, PG], F32)
        nc.gpsimd.memset(ONESB[:], 0.0)
        nc.gpsimd.memset(ONESB[0:N, 0:N], 1.0)
        nc.gpsimd.memset(ONESB[B1 : B1 + N, B1 : B1 + N], 1.0)
        # x0 columns: X0[0:49, p] = x0[2p], X0[64:113, p] = x0[2p+1].
        # Loaded via a contiguous [32, 49] DMA + PE transpose: a direct
        # strided DMA would burn ~0.7us of 4-byte descriptors on the DMA
        # engines ahead of the x stream.
        X0 = const.tile([PG, NPAIR], F32)
        nc.gpsimd.memset(X0[:], 0.0)
        IDT = const.tile([32, 32], F32)
        nc.gpsimd.memset(IDT[:], 0.0)
        nc.gpsimd.affine_select(
            out=IDT[:], in_=IDT[:], compare_op=ALU.not_equal, fill=1.0,
            base=0, pattern=[[-1, 32]], channel_multiplier=1,
        )
        S0 = const.tile([32, N], F32)
        # block-diagonal Gram matrices (off-block stays zero)
        Aall = const.tile([PG, NPAIR, PG], F32)
        nc.gpsimd.memset(Aall[:], 0.0)
        # squared Grams: G2 drives the B^2 double waves of run2 for every
        # chain and the 6-wave run1 of chain 0; G4 (tail pairs only) gives
        # chain 1 a 4-wave run1 (x9 = G4 G4 G x0)
        G2all = const.tile([PG, NPAIR, PG], F32)
        G4all = const.tile([PG, GP, PG], F32)

        # per-sample DMA view: partition q holds c-rows {512b + 4q + r},
        # 784B-contiguous descriptors (full DMA bandwidth); the (b, r)
        # enumeration of contraction tiles is a permutation of c, which
        # the Gram sum is invariant to.
        xrs = x.rearrange("s (b q r) j -> s q b (r j)", b=4, q=128, r=4)

        # DVE converts at 0.5 cyc/elem (2x mode), Act at 1/1.2GHz, Pool at
        # 1/(1.2GHz*0.6).  DVE takes the early evens (it is chain-free until
        # ~22us), Act the early odds plus the tail pair (it is free when the
        # tail arrives), Pool the middle stretch.
        # tail pairs (14-15) convert on DVE/Act right after chain 0's
        # engine blocks clear - their conv+gram prep is the gate into the
        # tail chain, so it gets the fastest converters.
        FP32_PAIRS = set()
        _CONV = {}
        for _idx in range(2 * NPAIR):
            if _idx <= 15:
                _CONV[_idx] = "dve" if _idx % 2 == 0 else "act"
            elif _idx >= 28:
                _CONV[_idx] = {28: "dve", 29: "pool", 30: "dve", 31: "pool"}[_idx]
            else:
                _CONV[_idx] = "pool"

        def conv_eng(idx):
            return {"dve": nc.vector, "act": nc.scalar, "pool": nc.gpsimd}[
                _CONV[idx]
            ]

        xt_tiles = {}
        xh_tiles = {}
        pair_ps = {}
        pending = []  # deferred half-sample Gram emitters, pumped into chains
        pen_dmas = []

        def emit_dma(p, nsub=1):
            for s in range(2):
                idx = 2 * p + s
                xt = xpool.tile([PG, KT * N], F32, tag="xt", bufs=8, name=f"xt{idx}")
                xv = xt.rearrange("q (b m) -> q b m", b=4)
                for u in range(nsub):
                    lo, hi = u * 4 // nsub, (u + 1) * 4 // nsub
                    nc.sync.dma_start(xv[:, lo:hi], xrs[idx, :, lo:hi])
                xt_tiles[idx] = xt
                if idx == 0:
                    nc.sync.dma_start(S0[:], x0)
                    pst = pspool.tile([N, 32], F32, tag="ray", bufs=1, name="pst")
                    nc.tensor.transpose(pst[:], S0[:], IDT[:])
                    pst_r = pst.rearrange("j (p e) -> e j p", e=2)
                    nc.vector.tensor_copy(X0[0:N, :], pst_r[0])
                    nc.vector.tensor_copy(X0[B1 : B1 + N, :], pst_r[1])

        def _copy(eng, out, in_):
            if eng is nc.scalar:
                eng.copy(out, in_)
            else:
                eng.tensor_copy(out, in_)

        def emit_conv(p):
            halves = 2 if p >= NPAIR - 2 else 1
            for s in range(2):
                idx = 2 * p + s
                xh = hpool.tile([PG, KT * N], F16, tag="xh", name=f"xh{idx}")
                half = KT * N // 2
                for u in range(halves):
                    lo, hi = (u * half, (u + 1) * half) if halves == 2 else (0, KT * N)
                    _copy(conv_eng(idx), xh[:, lo:hi], xt_tiles[idx][:, lo:hi])
                xh_tiles[idx] = xh

        def gram_job(p, s, h=None):
            # h=None: the whole sample (16 tiles, ~0.33us fp16); h=0/1: one
            # half - used for the fp32 tail pairs so their (4x costlier)
            # matmuls pipeline with the per-half sub-DMAs
            def job():
                if s == 0 and h in (None, 0):
                    pair_ps[p] = pspool.tile(
                        [PG, N], F32, tag="gram", bufs=3, name=f"pg{p}"
                    )
                pg_ = pair_ps[p]
                src = xt_tiles[2 * p + s] if p in FP32_PAIRS else xh_tiles[2 * p + s]
                ob = 0 if s == 0 else B1
                ks = range(KT) if h is None else range(8 * h, 8 * h + 8)
                for k in ks:
                    wk = src[:, k * N : (k + 1) * N]
                    nc.tensor.matmul(
                        pg_[ob : ob + N, :], wk, wk,
                        start=(k == 0), stop=(k == KT - 1),
                    )
            return job

        def copy_job(p, s):
            def job():
                pg_ = pair_ps[p]
                if s == 0:
                    nc.scalar.copy(Aall[0:N, p, 0:N], pg_[0:N, :])
                else:
                    nc.scalar.copy(
                        Aall[B1 : B1 + N, p, B1 : B1 + N], pg_[B1 : B1 + N, :]
                    )
            return job

        def sq_job(p, stage, part):
            # stage 0: G2 = A*A, stage 1: G4 = G2*G2; part 0 = PE matmul,
            # part 1 = PSUM->SBUF copy (DVE mid-stream; Act for the tail
            # pairs, since DVE is still busy with chain 0 when they land)
            def job():
                src = Aall[:, p, :] if stage == 0 else G2all[:, p, :]
                dst = G2all[:, p, :] if stage == 0 else G4all[:, p - GP, :]
                if part == 0:
                    ps2 = pspool.tile(
                        [PG, PG], F32, tag="sq", bufs=2, name=f"sq{p}_{stage}"
                    )
                    pair_ps[("sq", p)] = ps2
                    nc.tensor.matmul(ps2[:], src, src, start=True, stop=True)
                elif p >= NPAIR - 2:
                    nc.scalar.copy(dst, pair_ps.pop(("sq", p))[:])
                else:
                    nc.vector.tensor_copy(dst, pair_ps.pop(("sq", p))[:])
            return job

        # Jobs are (pair, thunk).  pump() only feeds jobs whose pair's data
        # lands while chain 0 is running (pairs < PUMP_CUTOFF); later pairs
        # would stall the chain waves on their DMA, so they run in the
        # post-chain-0 engine-idle window instead (via flush).
        PUMP_CUTOFF = NPAIR - 2

        def pump(k=1):
            for _ in range(k):
                if pending and pending[0][0] < PUMP_CUTOFF:
                    pending.pop(0)[1]()

        def flush():
            while pending:
                pending.pop(0)[1]()

        def queue_pair(p, tail=False):
            emit_dma(p, nsub=2 if p in FP32_PAIRS else 1)
            if p not in FP32_PAIRS:
                emit_conv(p)
            for s in range(2):
                if p in FP32_PAIRS:
                    pending.append((p, gram_job(p, s, 0)))
                    pending.append((p, gram_job(p, s, 1)))
                else:
                    pending.append((p, gram_job(p, s)))
            if tail:
                # tail-chain pairs: A copies join the pumped job stream;
                # G2 squarings for the last pumped pairs wait until after
                # the tail-pair prep (they are only needed by run2)
                pending.append((p, copy_job(p, 0)))
                pending.append((p, copy_job(p, 1)))
                if p < 3 * NPAIR // 4:
                    pending.append((p, sq_job(p, 0, 0)))
                    pending.append((p, sq_job(p, 0, 1)))

        def emit_copies(ps_list):
            for p in ps_list:
                copy_job(p, 0)()
                copy_job(p, 1)()

        pump_k = 4

        def emit_chain(g, run1="g2"):
            p0 = g * GP
            As = [Aall[:, p0 + j, :] for j in range(GP)]
            G2s = [G2all[:, p0 + j, :] for j in range(GP)]

            # ---- run1 matvec waves, fixups on Act (scalar engine).
            # Iterates stay unnormalized (power iteration is scale
            # invariant); the x9 fixup folds in 2^-52 so the Rayleigh
            # products stay in fp32 range.  Squared Grams shorten the
            # serial PSUM round-trip chain: x9 = G4 G4 G x0 (tail chain)
            # or G2^4 G x0 (chain 0, G4 copies not worth its window).
            if run1 == "g4":
                G4s = [G4all[:, j, :] for j in range(GP)]
                seq = [(As, "v"), (G4s, "v"), (G4s, "x9"), (As, "ray")]
            elif run1 == "g2":
                seq = [(As, "v")] + [(G2s, "v")] * 3 + [(G2s, "x9"), (As, "ray")]
            else:
                seq = [(As, "v")] * (NITER - 1) + [(As, "x9"), (As, "ray")]
            cur = X0[:, p0 : p0 + GP]
            x9s = None
            psw = None
            for i, (mats, kind) in enumerate(seq):
                psw = pspool.tile([PG, GP], F32, tag="mv", bufs=2, name=f"m1_{g}_{i}")
                for j in range(GP):
                    nc.tensor.matmul(
                        psw[:, j : j + 1], mats[j], cur[:, j : j + 1],
                        start=True, stop=True,
                    )
                pump(pump_k)
                if kind == "x9":
                    x9s = vpool.tile([PG, GP], F32, tag="x9", bufs=2, name=f"x9s{g}")
                    if g == NG - 1:
                        nc.vector.tensor_scalar(x9s[:], psw[:], S52, None, op0=ALU.mult)
                    else:
                        nc.scalar.mul(x9s[:], psw[:], S52)
                    cur = x9s[:]
                elif kind == "v":
                    nxt = vpool.tile([PG, GP], F32, tag="v", name=f"v1_{g}_{i}")
                    if g == NG - 1:
                        nc.vector.tensor_copy(nxt[:], psw[:])
                    else:
                        nc.scalar.copy(nxt[:], psw[:])
                    cur = nxt[:]

            # ---- Rayleigh 1 -> lambda in block layout (DVE) ------------
            # num/den = blockwise partition sums of x9*w and x9*x9 via the
            # block-diagonal-ones stationary: result lands broadcast on
            # every partition of the owning block.
            T = vpool.tile([PG, 2 * GP], F32, tag="T", bufs=2, name=f"T1_{g}")
            nc.vector.tensor_tensor(T[:, 0:GP], x9s[:], psw[:], op=ALU.mult)
            nc.vector.tensor_tensor(T[:, GP : 2 * GP], x9s[:], x9s[:], op=ALU.mult)
            pnd = pspool.tile([PG, 2 * GP], F32, tag="ray", bufs=1, name=f"pn1_{g}")
            nc.tensor.matmul(pnd[:], ONESB[:], T[:], start=True, stop=True)
            pump(pump_k)
            # +tiny while copying out of PSUM: on the junk partitions
            # outside the 49-blocks num = den = 0, and a bare reciprocal
            # would give lam = 0*inf = NaN there, which the fused run2
            # waves would then propagate into the blocks (0*NaN = NaN in
            # the matvec contraction).  With the bias those rows give
            # lam = tiny/tiny = 1.0, which multiplies the zero iterate
            # harmlessly.  On block rows den is ~1e29+, so the bias is
            # far below one ulp.
            ndc = vpool.tile([PG, 2 * GP], F32, tag="nd", bufs=2, name=f"nd1_{g}")
            nc.vector.tensor_scalar(ndc[:], pnd[:], 1e-30, None, op0=ALU.add)
            rd = vpool.tile([PG, GP], F32, tag="rd", bufs=2, name=f"rd{g}")
            nc.vector.reciprocal(rd[:], ndc[:, GP : 2 * GP])
            LAMV = vpool.tile([PG, GP], F32, tag="lam", bufs=2, name=f"lam{g}")
            nc.vector.tensor_tensor(LAMV[:], ndc[:, 0:GP], rd[:], op=ALU.mult)
            LAMV2 = vpool.tile([PG, GP], F32, tag="lam2", bufs=2, name=f"l2_{g}")
            nc.vector.tensor_scalar(LAMV2[:], LAMV[:], 2.0, None, op0=ALU.mult)
            LAMSQ = vpool.tile([PG, GP], F32, tag="lamsq", bufs=2, name=f"lq_{g}")
            nc.vector.tensor_tensor(LAMSQ[:], LAMV[:], LAMV[:], op=ALU.mult)
            x1 = vpool.tile([PG, GP], F32, tag="x1", bufs=2, name=f"x1_{g}")
            nc.scalar.mul(x1[:], x9s[:], S104)

            # ---- run2: fused shifted waves, B v = G v - lam*v.  The
            # lam*cur (and lam^2*cur) products are issued before the matvec
            # so they hide under it; fixups run on DVE.  The tail chain
            # applies B^2 = G2 - 2*lam*G + lam^2 per wave, halving the
            # number of serial PSUM round trips.
            cur = x1[:]
            u9 = None
            steps = ["s"] + ["d"] * 4
            u9_at = 4
            for i, kind in enumerate(steps):
                if kind == "s":
                    t = vpool.tile([PG, GP], F32, tag="v", name=f"t2_{g}_{i}")
                    nc.vector.tensor_tensor(t[:], LAMV[:], cur, op=ALU.mult)
                    psw = pspool.tile(
                        [PG, GP], F32, tag="mv", bufs=2, name=f"m2_{g}_{i}"
                    )
                    for j in range(GP):
                        nc.tensor.matmul(
                            psw[:, j : j + 1], As[j], cur[:, j : j + 1],
                            start=True, stop=True,
                        )
                    pump(pump_k)
                    nxt = vpool.tile([PG, GP], F32, tag="v", name=f"v2_{g}_{i}")
                    nc.vector.tensor_tensor(nxt[:], psw[:], t[:], op=ALU.subtract)
                else:
                    e = vpool.tile([PG, GP], F32, tag="v", name=f"e2_{g}_{i}")
                    nc.vector.tensor_tensor(e[:], LAMSQ[:], cur, op=ALU.mult)
                    psa = pspool.tile(
                        [PG, GP], F32, tag="mv", bufs=2, name=f"m2a_{g}_{i}"
                    )
                    psb = pspool.tile(
                        [PG, GP], F32, tag="mv", bufs=2, name=f"m2b_{g}_{i}"
                    )
                    for j in range(GP):
                        nc.tensor.matmul(
                            psa[:, j : j + 1], G2s[j], cur[:, j : j + 1],
                            start=True, stop=True,
                        )
                    for j in range(GP):
                        nc.tensor.matmul(
                            psb[:, j : j + 1], As[j], cur[:, j : j + 1],
                            start=True, stop=True,
                        )
                    # the two PSUM reads are independent so their access
                    # latencies overlap; only the final sbuf-sbuf subtract
                    # waits on both
                    t1 = vpool.tile([PG, GP], F32, tag="v", name=f"t1_{g}_{i}")
                    nc.vector.tensor_tensor(t1[:], psa[:], e[:], op=ALU.add)
                    d1 = vpool.tile([PG, GP], F32, tag="v", name=f"d1_{g}_{i}")
                    nc.vector.tensor_tensor(d1[:], LAMV2[:], psb[:], op=ALU.mult)
                    nxt = vpool.tile([PG, GP], F32, tag="v", name=f"v2_{g}_{i}")
                    nc.vector.tensor_tensor(nxt[:], t1[:], d1[:], op=ALU.subtract)
                if i == u9_at:
                    u9 = nxt
                cur = nxt[:]
            # final application is a bare matvec: u9^T B u9 = u9.psw - lam*d2,
            # so the shifted fixup cancels out of the penalty algebraically
            psw = pspool.tile([PG, GP], F32, tag="mv", bufs=2, name=f"m2w_{g}")
            for j in range(GP):
                nc.tensor.matmul(
                    psw[:, j : j + 1], As[j], cur[:, j : j + 1],
                    start=True, stop=True,
                )
            # remaining queued tail-pair prep: emitted here (PE idle gaps
            # between this chain's end and the tail chain) rather than
            # behind this chain's R2 matmul
            flush()

            # ---- Rayleigh 2 + penalty ---------------------------------
            # n2'' = u9.(G u9), d2 = u9.u9; tmp + lam = n2''/d2, so
            # pen = (tmp/(tmp+lam))^2 = ((n2'' - lam*d2) / n2'')^2.
            # recip goes first so its PSUM access latency overlaps t2v's.
            T2 = vpool.tile([PG, 2 * GP], F32, tag="T", bufs=2, name=f"T2_{g}")
            nc.vector.tensor_tensor(T2[:, GP : 2 * GP], u9[:], u9[:], op=ALU.mult)
            nc.vector.tensor_tensor(T2[:, 0:GP], u9[:], psw[:], op=ALU.mult)
            pnd2 = pspool.tile([PG, 2 * GP], F32, tag="ray", bufs=1, name=f"pn2_{g}")
            nc.tensor.matmul(pnd2[:], ONESB[:], T2[:], start=True, stop=True)
            rq = vpool.tile([PG, GP], F32, tag="rq", bufs=2, name=f"rq{g}")
            nc.vector.reciprocal(rq[:], pnd2[:, 0:GP])
            t2v = vpool.tile([PG, GP], F32, tag="rd", bufs=2, name=f"t2v{g}")
            nc.vector.tensor_tensor(t2v[:], LAMV[:], pnd2[:, GP : 2 * GP], op=ALU.mult)
            n2x = vpool.tile([PG, GP], F32, tag="q", bufs=2, name=f"n2x{g}")
            nc.vector.tensor_tensor(n2x[:], pnd2[:, 0:GP], t2v[:], op=ALU.subtract)
            rt = vpool.tile([PG, GP], F32, tag="rt", bufs=2, name=f"rt{g}")
            nc.vector.tensor_tensor(rt[:], n2x[:], rq[:], op=ALU.mult)
            PENg = vpool.tile([PG, GP], F32, tag="pen", bufs=2, name=f"pen{g}")
            nc.vector.tensor_tensor(PENg[:], rt[:], rt[:], op=ALU.mult)

            # pen[8g + 2j + e] = PENg[64e, j]; deferred to SP after all x
            # DMAs so no engine's stream queues behind a chain-gated DMA.
            pen_r = pen.rearrange("(g j e) -> g e j", g=NG, e=2)
            PEN_v = PENg.rearrange("(b q) p -> b q p", b=2)[:, 0, :]
            pen_dmas.append((pen_r[g], PEN_v))

        groups = [list(range(g * GP, (g + 1) * GP)) for g in range(NG)]

        TAIL = (NPAIR - 2, NPAIR - 1)
        for p in groups[0]:
            queue_pair(p)
        flush()
        emit_copies(groups[0])
        for p in groups[1]:
            if p in TAIL:
                emit_dma(p, nsub=2 if p == NPAIR - 1 else 1)
            else:
                queue_pair(p, tail=True)
        for p in groups[0]:
            sq_job(p, 0, 0)()
            sq_job(p, 0, 1)()
        emit_chain(0, run1="g2")
        flush()
        # tail-pair prep lands in the post-chain-0 window where PE, DVE
        # and Act are all otherwise idle
        # tail prep is the serial gate into chain 1: split every step
        # across DVE and Act so the two engines halve it between them
        for p in TAIL:
            emit_conv(p)
            if p == NPAIR - 1:
                gram_job(p, 0)()
                gram_job(p, 1, 0)()
                gram_job(p, 1, 1)()
            else:
                gram_job(p, 0)()
                gram_job(p, 1)()
        # all four A copies ahead of any squaring (wave 1 only needs A;
        # G2 is not used until run2), block-a on DVE, block-b on Act
        for p in TAIL:
            pg_ = pair_ps.pop(p)
            nc.vector.tensor_copy(Aall[0:N, p, 0:N], pg_[0:N, :])
            nc.vector.tensor_copy(Aall[B1 : B1 + N, p, B1 : B1 + N], pg_[B1 : B1 + N, :])
        for p in TAIL:
            sq_job(p, 0, 0)()
            sq_job(p, 0, 1)()
        for p in range(3 * NPAIR // 4, NPAIR - 2):
            sq_job(p, 0, 0)()
            sq_job(p, 0, 1)()
        emit_chain(1, run1="g2")
        for dst, src in pen_dmas:
            nc.sync.dma_start(dst, src)


_NC_CACHE = {}


def build_nc(repeat=1):
    if repeat in _NC_CACHE:
        return _NC_CACHE[repeat]
    nc = bacc.Bacc("TRN2", target_bir_lowering=False, debug=False)
    x = nc.dram_tensor("x", [BS, C, N], F32, kind="ExternalInput")
    x0 = nc.dram_tensor("x0", [BS, N], F32, kind="ExternalInput")
    pen = nc.dram_tensor("pen", [BS], F32, kind="ExternalOutput")
    with tile.TileContext(nc) as tc:
        _emit(tc, x.ap(), x0.ap(), pen.ap())
    nc.compile()
    _NC_CACHE[repeat] = nc
    return nc


LAST_RESULTS = None


def kernel(x, x0):
    global LAST_RESULTS
    x = np.ascontiguousarray(np.asarray(x, dtype=np.float32).reshape(B, C, N))
    x0 = np.ascontiguousarray(np.asarray(x0, dtype=np.float32).reshape(B, N))
    nc = build_nc()
    in_maps = [
        {"x": x[i * BS : (i + 1) * BS], "x0": x0[i * BS : (i + 1) * BS]}
        for i in range(NCORES)
    ]
    trace = bool(int(os.environ.get("KERNEL_TRACE", "0")))
    res = run_bass_kernel_spmd(nc, in_maps, list(range(NCORES)), trace=trace)
    LAST_RESULTS = res
    pens = np.concatenate([r["pen"].reshape(-1) for r in res.results])
    return np.float32(pens.sum(dtype=np.float64) / B)


# revision 87
# speedup vs baseline: 1.3388x; 1.0034x over previous
"""Trainium2 Bass kernel for the OFPenalty eigenvalue-penalty loss.

Math (per sample b of 256):
  W = x[b] reshaped [C=2048, N=49];  G = W^T W  (49x49 Gram matrix)
  run1: x9 = G^9 x0 (power iteration, normalization deferred - scale
        invariant), largest = Rayleigh(G, x9) = x9^T G x9 / x9^T x9
  run2: B = G - largest*I applied fused per wave (never materialized),
        u9 = B^9 x1 (x1 = scaled x9), tmp = Rayleigh(B, u9)
  penalty = (tmp/(tmp+largest))^2 ; output = mean over batch.

Layout: pure data parallel, 32 samples per core on 8 cores.  Samples
are packed in pairs block-diagonally: sample 2p on partitions 0:49,
sample 2p+1 on 64:113.  The Gram inputs are converted fp32->fp16 so
the 512 Gram matmuls run at 1 cycle/row instead of 4 (PSUM still
accumulates fp32; rel err stays ~1e-4, far under the 2e-2 gate).

Pipelining: the 16 pairs stream in DMA order in two groups of 8.
Group 0's eigen-chain (run1, Rayleigh/lambda, fused run2, penalty)
runs while group 1's DMA + conversion + Grams continue; group 1's prep
jobs are pumped into the engine gaps between chain-0 waves, and only
the tail pair's prep plus chain 1 trail the final DMA.  Serial PSUM
round trips are halved with squared Grams: run1 applies G2 (x9 =
G2^4 G x0, 6 waves) and run2 applies B^2 = G2 - 2*lam*G + lam^2
per double wave (B^9 = B^2^4 B, 6 waves).

Rayleigh sums use a block-diagonal-ones stationary so the per-sample
numerator/denominator land broadcast across all partitions in block
layout - lambda feeds the fused run2 with no rank-1 rebroadcast.
Iterate overflow is handled by folding exact powers-of-two scales into
single fixup/scale ops (wave 9 of run1, and the run2 warm start).
"""

import os
import sys
from contextlib import ExitStack

import numpy as np

for _p in ("/opt/trn_rl_repo",):
    if os.path.isdir(_p) and _p not in sys.path:
        sys.path.insert(0, _p)

import concourse.bass as bass  # noqa: E402
import concourse.tile as tile  # noqa: E402
from concourse import bacc, mybir  # noqa: E402
from concourse.bass_utils import run_bass_kernel_spmd  # noqa: E402

F32 = mybir.dt.float32
F16 = mybir.dt.float16
ALU = mybir.AluOpType

B, C, N = 256, 2048, 49
NCORES = 8
BS = B // NCORES  # 32 samples per core
NPAIR = BS // 2  # 16 pairs
KT = C // 128  # 16 contraction tiles
PG = 128  # gapped pair-vector space: blocks at [0:49], [64:113]
B1 = 64  # partition base of the second sample in a pair
NG = 2  # pipeline groups
GP = NPAIR // NG  # pairs per group
NITER = 9
S52 = float(2.0**-52)  # rescale x9 before Rayleigh products
S104 = float(2.0**-104)  # rescale x9 -> x1 (run2 warm start)


def _emit(tc, x, x0, pen):
    nc = tc.nc
    ctx = ExitStack()
    with ctx:
        const = ctx.enter_context(tc.tile_pool(name="const", bufs=1))
        xpool = ctx.enter_context(tc.tile_pool(name="xt", bufs=8))
        hpool = ctx.enter_context(tc.tile_pool(name="xh", bufs=16))
        vpool = ctx.enter_context(tc.tile_pool(name="vec", bufs=6))
        pspool = ctx.enter_context(tc.tile_pool(name="ps", bufs=1, space="PSUM"))

        # ---- constants -------------------------------------------------
        # block-diagonal ones: partition-sum broadcast within each block
        ONESB = const.tile([PG, PG], F32)
        nc.gpsimd.memset(ONESB[:], 0.0)
        nc.gpsimd.memset(ONESB[0:N, 0:N], 1.0)
        nc.gpsimd.memset(ONESB[B1 : B1 + N, B1 : B1 + N], 1.0)
        # x0 columns: X0[0:49, p] = x0[2p], X0[64:113, p] = x0[2p+1].
        # Loaded via a contiguous [32, 49] DMA + PE transpose: a direct
        # strided DMA would burn ~0.7us of 4-byte descriptors on the DMA
        # engines ahead of the x stream.
        X0 = const.tile([PG, NPAIR], F32)
        nc.gpsimd.memset(X0[:], 0.0)
        IDT = const.tile([32, 32], F32)
        nc.gpsimd.memset(IDT[:], 0.0)
        nc.gpsimd.affine_select(
            out=IDT[:], in_=IDT[:], compare_op=ALU.not_equal, fill=1.0,
            base=0, pattern=[[-1, 32]], channel_multiplier=1,
        )
        S0 = const.tile([32, N], F32)
        # block-diagonal Gram matrices (off-block stays zero)
        Aall = const.tile([PG, NPAIR, PG], F32)
        nc.gpsimd.memset(Aall[:], 0.0)
        # squared Grams: G2 drives the B^2 double waves of run2 for every
        # chain and the 6-wave run1 of chain 0; G4 (tail pairs only) gives
        # chain 1 a 4-wave run1 (x9 = G4 G4 G x0)
        G2all = const.tile([PG, NPAIR, PG], F32)
        G4all = const.tile([PG, GP, PG], F32)

        # per-sample DMA view: partition q holds c-rows {512b + 4q + r},
        # 784B-contiguous descriptors (full DMA bandwidth); the (b, r)
        # enumeration of contraction tiles is a permutation of c, which
        # the Gram sum is invariant to.
        xrs = x.rearrange("s (b q r) j -> s q b (r j)", b=4, q=128, r=4)

        # DVE converts at 0.5 cyc/elem (2x mode), Act at 1/1.2GHz, Pool at
        # 1/(1.2GHz*0.6).  DVE takes the early evens (it is chain-free until
        # ~22us), Act the early odds plus the tail pair (it is free when the
        # tail arrives), Pool the middle stretch.
        # tail pairs (14-15) convert on DVE/Act right after chain 0's
        # engine blocks clear - their conv+gram prep is the gate into the
        # tail chain, so it gets the fastest converters.
        FP32_PAIRS = set()
        _CONV = {}
        for _idx in range(2 * NPAIR):
            if _idx <= 13:
                _CONV[_idx] = "dve" if _idx % 2 == 0 else "act"
            elif _idx >= 28:
                _CONV[_idx] = {28: "dve", 29: "pool", 30: "dve", 31: "pool"}[_idx]
            else:
                _CONV[_idx] = "pool"

        def conv_eng(idx):
            return {"dve": nc.vector, "act": nc.scalar, "pool": nc.gpsimd}[
                _CONV[idx]
            ]

        xt_tiles = {}
        xh_tiles = {}
        pair_ps = {}
        pending = []  # deferred half-sample Gram emitters, pumped into chains
        pen_dmas = []

        def emit_dma(p, nsub=1):
            for s in range(2):
                idx = 2 * p + s
                xt = xpool.tile([PG, KT * N], F32, tag="xt", bufs=8, name=f"xt{idx}")
                xv = xt.rearrange("q (b m) -> q b m", b=4)
                for u in range(nsub):
                    lo, hi = u * 4 // nsub, (u + 1) * 4 // nsub
                    nc.sync.dma_start(xv[:, lo:hi], xrs[idx, :, lo:hi])
                xt_tiles[idx] = xt
                if idx == 0:
                    nc.sync.dma_start(S0[:], x0)
                    pst = pspool.tile([N, 32], F32, tag="ray", bufs=1, name="pst")
                    nc.tensor.transpose(pst[:], S0[:], IDT[:])
                    pst_r = pst.rearrange("j (p e) -> e j p", e=2)
                    nc.vector.tensor_copy(X0[0:N, :], pst_r[0])
                    nc.vector.tensor_copy(X0[B1 : B1 + N, :], pst_r[1])

        def _copy(eng, out, in_):
            if eng is nc.scalar:
                eng.copy(out, in_)
            else:
                eng.tensor_copy(out, in_)

        def emit_conv(p):
            halves = 2 if p >= NPAIR - 2 else 1
            for s in range(2):
                idx = 2 * p + s
                xh = hpool.tile([PG, KT * N], F16, tag="xh", name=f"xh{idx}")
                half = KT * N // 2
                for u in range(halves):
                    lo, hi = (u * half, (u + 1) * half) if halves == 2 else (0, KT * N)
                    _copy(conv_eng(idx), xh[:, lo:hi], xt_tiles[idx][:, lo:hi])
                xh_tiles[idx] = xh

        def gram_job(p, s, h=None):
            # h=None: the whole sample (16 tiles, ~0.33us fp16); h=0/1: one
            # half - used for the fp32 tail pairs so their (4x costlier)
            # matmuls pipeline with the per-half sub-DMAs
            def job():
                if s == 0 and h in (None, 0):
                    pair_ps[p] = pspool.tile(
                        [PG, N], F32, tag="gram", bufs=2, name=f"pg{p}"
                    )
                pg_ = pair_ps[p]
                src = xt_tiles[2 * p + s] if p in FP32_PAIRS else xh_tiles[2 * p + s]
                ob = 0 if s == 0 else B1
                ks = range(KT) if h is None else range(8 * h, 8 * h + 8)
                for k in ks:
                    wk = src[:, k * N : (k + 1) * N]
                    nc.tensor.matmul(
                        pg_[ob : ob + N, :], wk, wk,
                        start=(k == 0), stop=(k == KT - 1),
                    )
            return job

        def copy_job(p, s):
            def job():
                pg_ = pair_ps[p]
                if s == 0:
                    nc.scalar.copy(Aall[0:N, p, 0:N], pg_[0:N, :])
                else:
                    nc.scalar.copy(
                        Aall[B1 : B1 + N, p, B1 : B1 + N], pg_[B1 : B1 + N, :]
                    )
            return job

        def sq_job(p, stage, part):
            # stage 0: G2 = A*A, stage 1: G4 = G2*G2; part 0 = PE matmul,
            # part 1 = PSUM->SBUF copy (DVE mid-stream; Act for the tail
            # pairs, since DVE is still busy with chain 0 when they land)
            def job():
                src = Aall[:, p, :] if stage == 0 else G2all[:, p, :]
                dst = G2all[:, p, :] if stage == 0 else G4all[:, p - GP, :]
                if part == 0:
                    ps2 = pspool.tile(
                        [PG, PG], F32, tag="sq", bufs=3, name=f"sq{p}_{stage}"
                    )
                    pair_ps[("sq", p)] = ps2
                    nc.tensor.matmul(ps2[:], src, src, start=True, stop=True)
                elif p >= NPAIR - 2:
                    nc.scalar.copy(dst, pair_ps.pop(("sq", p))[:])
                else:
                    nc.vector.tensor_copy(dst, pair_ps.pop(("sq", p))[:])
            return job

        # Jobs are (pair, thunk).  pump() only feeds jobs whose pair's data
        # lands while chain 0 is running (pairs < PUMP_CUTOFF); later pairs
        # would stall the chain waves on their DMA, so they run in the
        # post-chain-0 engine-idle window instead (via flush).
        PUMP_CUTOFF = NPAIR - 2

        def pump(k=1):
            for _ in range(k):
                if pending and pending[0][0] < PUMP_CUTOFF:
                    pending.pop(0)[1]()

        def flush():
            while pending:
                pending.pop(0)[1]()

        def queue_pair(p, tail=False):
            emit_dma(p, nsub=2 if p in FP32_PAIRS else 1)
            if p not in FP32_PAIRS:
                emit_conv(p)
            for s in range(2):
                if p in FP32_PAIRS:
                    pending.append((p, gram_job(p, s, 0)))
                    pending.append((p, gram_job(p, s, 1)))
                else:
                    pending.append((p, gram_job(p, s)))
            if tail:
                # tail-chain pairs: A copies join the pumped job stream;
                # G2 squarings for the last pumped pairs wait until after
                # the tail-pair prep (they are only needed by run2)
                pending.append((p, copy_job(p, 0)))
                pending.append((p, copy_job(p, 1)))
                if p < 3 * NPAIR // 4:
                    pending.append((p, sq_job(p, 0, 0)))
                    pending.append((p, sq_job(p, 0, 1)))

        def emit_copies(ps_list):
            for p in ps_list:
                copy_job(p, 0)()
                copy_job(p, 1)()

        pump_k = 4

        def emit_chain(g, run1="g2"):
            p0 = g * GP
            As = [Aall[:, p0 + j, :] for j in range(GP)]
            G2s = [G2all[:, p0 + j, :] for j in range(GP)]

            # ---- run1 matvec waves, fixups on Act (scalar engine).
            # Iterates stay unnormalized (power iteration is scale
            # invariant); the x9 fixup folds in 2^-52 so the Rayleigh
            # products stay in fp32 range.  Squared Grams shorten the
            # serial PSUM round-trip chain: x9 = G4 G4 G x0 (tail chain)
            # or G2^4 G x0 (chain 0, G4 copies not worth its window).
            if run1 == "g4":
                G4s = [G4all[:, j, :] for j in range(GP)]
                seq = [(As, "v"), (G4s, "v"), (G4s, "x9"), (As, "ray")]
            elif run1 == "g2":
                seq = [(As, "v")] + [(G2s, "v")] * 3 + [(G2s, "x9"), (As, "ray")]
            else:
                seq = [(As, "v")] * (NITER - 1) + [(As, "x9"), (As, "ray")]
            cur = X0[:, p0 : p0 + GP]
            x9s = None
            psw = None
            for i, (mats, kind) in enumerate(seq):
                psw = pspool.tile([PG, GP], F32, tag="mv", bufs=2, name=f"m1_{g}_{i}")
                for j in range(GP):
                    nc.tensor.matmul(
                        psw[:, j : j + 1], mats[j], cur[:, j : j + 1],
                        start=True, stop=True,
                    )
                pump(pump_k)
                if kind == "x9":
                    x9s = vpool.tile([PG, GP], F32, tag="x9", bufs=2, name=f"x9s{g}")
                    if g == NG - 1:
                        nc.vector.tensor_scalar(x9s[:], psw[:], S52, None, op0=ALU.mult)
                    else:
                        nc.scalar.mul(x9s[:], psw[:], S52)
                    cur = x9s[:]
                elif kind == "v":
                    nxt = vpool.tile([PG, GP], F32, tag="v", name=f"v1_{g}_{i}")
                    if g == NG - 1:
                        nc.vector.tensor_copy(nxt[:], psw[:])
                    else:
                        nc.scalar.copy(nxt[:], psw[:])
                    cur = nxt[:]

            # ---- Rayleigh 1 -> lambda in block layout (DVE) ------------
            # num/den = blockwise partition sums of x9*w and x9*x9 via the
            # block-diagonal-ones stationary: result lands broadcast on
            # every partition of the owning block.
            T = vpool.tile([PG, 2 * GP], F32, tag="T", bufs=2, name=f"T1_{g}")
            nc.vector.tensor_tensor(T[:, 0:GP], x9s[:], psw[:], op=ALU.mult)
            nc.vector.tensor_tensor(T[:, GP : 2 * GP], x9s[:], x9s[:], op=ALU.mult)
            pnd = pspool.tile([PG, 2 * GP], F32, tag="ray", bufs=1, name=f"pn1_{g}")
            nc.tensor.matmul(pnd[:], ONESB[:], T[:], start=True, stop=True)
            pump(pump_k)
            # +tiny while copying out of PSUM: on the junk partitions
            # outside the 49-blocks num = den = 0, and a bare reciprocal
            # would give lam = 0*inf = NaN there, which the fused run2
            # waves would then propagate into the blocks (0*NaN = NaN in
            # the matvec contraction).  With the bias those rows give
            # lam = tiny/tiny = 1.0, which multiplies the zero iterate
            # harmlessly.  On block rows den is ~1e29+, so the bias is
            # far below one ulp.
            ndc = vpool.tile([PG, 2 * GP], F32, tag="nd", bufs=2, name=f"nd1_{g}")
            nc.vector.tensor_scalar(ndc[:], pnd[:], 1e-30, None, op0=ALU.add)
            rd = vpool.tile([PG, GP], F32, tag="rd", bufs=2, name=f"rd{g}")
            nc.vector.reciprocal(rd[:], ndc[:, GP : 2 * GP])
            LAMV = vpool.tile([PG, GP], F32, tag="lam", bufs=2, name=f"lam{g}")
            nc.vector.tensor_tensor(LAMV[:], ndc[:, 0:GP], rd[:], op=ALU.mult)
            LAMV2 = vpool.tile([PG, GP], F32, tag="lam2", bufs=2, name=f"l2_{g}")
            nc.vector.tensor_scalar(LAMV2[:], LAMV[:], 2.0, None, op0=ALU.mult)
            LAMSQ = vpool.tile([PG, GP], F32, tag="lamsq", bufs=2, name=f"lq_{g}")
            nc.vector.tensor_tensor(LAMSQ[:], LAMV[:], LAMV[:], op=ALU.mult)
            x1 = vpool.tile([PG, GP], F32, tag="x1", bufs=2, name=f"x1_{g}")
            nc.scalar.mul(x1[:], x9s[:], S104)

            # ---- run2: fused shifted waves, B v = G v - lam*v.  The
            # lam*cur (and lam^2*cur) products are issued before the matvec
            # so they hide under it; fixups run on DVE.  The tail chain
            # applies B^2 = G2 - 2*lam*G + lam^2 per wave, halving the
            # number of serial PSUM round trips.
            cur = x1[:]
            u9 = None
            steps = ["s"] + ["d"] * 4
            u9_at = 4
            for i, kind in enumerate(steps):
                if kind == "s":
                    t = vpool.tile([PG, GP], F32, tag="v", name=f"t2_{g}_{i}")
                    nc.vector.tensor_tensor(t[:], LAMV[:], cur, op=ALU.mult)
                    psw = pspool.tile(
                        [PG, GP], F32, tag="mv", bufs=2, name=f"m2_{g}_{i}"
                    )
                    for j in range(GP):
                        nc.tensor.matmul(
                            psw[:, j : j + 1], As[j], cur[:, j : j + 1],
                            start=True, stop=True,
                        )
                    pump(pump_k)
                    nxt = vpool.tile([PG, GP], F32, tag="v", name=f"v2_{g}_{i}")
                    nc.vector.tensor_tensor(nxt[:], psw[:], t[:], op=ALU.subtract)
                else:
                    e = vpool.tile([PG, GP], F32, tag="v", name=f"e2_{g}_{i}")
                    nc.vector.tensor_tensor(e[:], LAMSQ[:], cur, op=ALU.mult)
                    psa = pspool.tile(
                        [PG, GP], F32, tag="mv", bufs=2, name=f"m2a_{g}_{i}"
                    )
                    psb = pspool.tile(
                        [PG, GP], F32, tag="mv", bufs=2, name=f"m2b_{g}_{i}"
                    )
                    for j in range(GP):
                        nc.tensor.matmul(
                            psa[:, j : j + 1], G2s[j], cur[:, j : j + 1],
                            start=True, stop=True,
                        )
                    for j in range(GP):
                        nc.tensor.matmul(
                            psb[:, j : j + 1], As[j], cur[:, j : j + 1],
                            start=True, stop=True,
                        )
                    # the two PSUM reads are independent so their access
                    # latencies overlap; only the final sbuf-sbuf subtract
                    # waits on both
                    t1 = vpool.tile([PG, GP], F32, tag="v", name=f"t1_{g}_{i}")
                    nc.vector.tensor_tensor(t1[:], psa[:], e[:], op=ALU.add)
                    d1 = vpool.tile([PG, GP], F32, tag="v", name=f"d1_{g}_{i}")
                    nc.vector.tensor_tensor(d1[:], LAMV2[:], psb[:], op=ALU.mult)
                    nxt = vpool.tile([PG, GP], F32, tag="v", name=f"v2_{g}_{i}")
                    nc.vector.tensor_tensor(nxt[:], t1[:], d1[:], op=ALU.subtract)
                if i == u9_at:
                    u9 = nxt
                cur = nxt[:]
            # final application is a bare matvec: u9^T B u9 = u9.psw - lam*d2,
            # so the shifted fixup cancels out of the penalty algebraically
            psw = pspool.tile([PG, GP], F32, tag="mv", bufs=2, name=f"m2w_{g}")
            for j in range(GP):
                nc.tensor.matmul(
                    psw[:, j : j + 1], As[j], cur[:, j : j + 1],
                    start=True, stop=True,
                )
            # remaining queued tail-pair prep: emitted here (PE idle gaps
            # between this chain's end and the tail chain) rather than
            # behind this chain's R2 matmul
            flush()

            # ---- Rayleigh 2 + penalty ---------------------------------
            # n2'' = u9.(G u9), d2 = u9.u9; tmp + lam = n2''/d2, so
            # pen = (tmp/(tmp+lam))^2 = ((n2'' - lam*d2) / n2'')^2.
            # recip goes first so its PSUM access latency overlaps t2v's.
            T2 = vpool.tile([PG, 2 * GP], F32, tag="T", bufs=2, name=f"T2_{g}")
            nc.vector.tensor_tensor(T2[:, GP : 2 * GP], u9[:], u9[:], op=ALU.mult)
            nc.vector.tensor_tensor(T2[:, 0:GP], u9[:], psw[:], op=ALU.mult)
            pnd2 = pspool.tile([PG, 2 * GP], F32, tag="ray", bufs=1, name=f"pn2_{g}")
            nc.tensor.matmul(pnd2[:], ONESB[:], T2[:], start=True, stop=True)
            rq = vpool.tile([PG, GP], F32, tag="rq", bufs=2, name=f"rq{g}")
            nc.vector.reciprocal(rq[:], pnd2[:, 0:GP])
            t2v = vpool.tile([PG, GP], F32, tag="rd", bufs=2, name=f"t2v{g}")
            nc.vector.tensor_tensor(t2v[:], LAMV[:], pnd2[:, GP : 2 * GP], op=ALU.mult)
            n2x = vpool.tile([PG, GP], F32, tag="q", bufs=2, name=f"n2x{g}")
            nc.vector.tensor_tensor(n2x[:], pnd2[:, 0:GP], t2v[:], op=ALU.subtract)
            rt = vpool.tile([PG, GP], F32, tag="rt", bufs=2, name=f"rt{g}")
            nc.vector.tensor_tensor(rt[:], n2x[:], rq[:], op=ALU.mult)
            PENg = vpool.tile([PG, GP], F32, tag="pen", bufs=2, name=f"pen{g}")
            nc.vector.tensor_tensor(PENg[:], rt[:], rt[:], op=ALU.mult)

            # pen[8g + 2j + e] = PENg[64e, j]; deferred to SP after all x
            # DMAs so no engine's stream queues behind a chain-gated DMA.
            pen_r = pen.rearrange("(g j e) -> g e j", g=NG, e=2)
            PEN_v = PENg.rearrange("(b q) p -> b q p", b=2)[:, 0, :]
            pen_dmas.append((pen_r[g], PEN_v))

        groups = [list(range(g * GP, (g + 1) * GP)) for g in range(NG)]

        TAIL = (NPAIR - 2, NPAIR - 1)
        for p in groups[0]:
            queue_pair(p)
        flush()
        emit_copies(groups[0])
        for p in groups[1]:
            if p in TAIL:
                emit_dma(p, nsub=2 if p == NPAIR - 1 else 1)
            else:
                queue_pair(p, tail=True)
        for p in groups[0]:
            sq_job(p, 0, 0)()
            sq_job(p, 0, 1)()
        emit_chain(0, run1="g2")
        flush()
        # tail-pair prep lands in the post-chain-0 window where PE, DVE
        # and Act are all otherwise idle
        # tail prep is the serial gate into chain 1: split every step
        # across DVE and Act so the two engines halve it between them
        for p in TAIL:
            emit_conv(p)
            if p == NPAIR - 1:
                gram_job(p, 0)()
                gram_job(p, 1, 0)()
                gram_job(p, 1, 1)()
            else:
                gram_job(p, 0)()
                gram_job(p, 1)()
        # all four A copies ahead of any squaring (wave 1 only needs A;
        # G2 is not used until run2), block-a on DVE, block-b on Act
        for p in TAIL:
            pg_ = pair_ps.pop(p)
            nc.vector.tensor_copy(Aall[0:N, p, 0:N], pg_[0:N, :])
            nc.vector.tensor_copy(Aall[B1 : B1 + N, p, B1 : B1 + N], pg_[B1 : B1 + N, :])
        for p in TAIL:
            sq_job(p, 0, 0)()
            sq_job(p, 0, 1)()
        for p in range(3 * NPAIR // 4, NPAIR - 2):
            sq_job(p, 0, 0)()
            sq_job(p, 0, 1)()
        emit_chain(1, run1="g2")
        for dst, src in pen_dmas:
            nc.sync.dma_start(dst, src)


_NC_CACHE = {}


def build_nc(repeat=1):
    if repeat in _NC_CACHE:
        return _NC_CACHE[repeat]
    nc = bacc.Bacc("TRN2", target_bir_lowering=False, debug=False)
    x = nc.dram_tensor("x", [BS, C, N], F32, kind="ExternalInput")
    x0 = nc.dram_tensor("x0", [BS, N], F32, kind="ExternalInput")
    pen = nc.dram_tensor("pen", [BS], F32, kind="ExternalOutput")
    with tile.TileContext(nc) as tc:
        _emit(tc, x.ap(), x0.ap(), pen.ap())
    nc.compile()
    _NC_CACHE[repeat] = nc
    return nc


LAST_RESULTS = None


def kernel(x, x0):
    global LAST_RESULTS
    x = np.ascontiguousarray(np.asarray(x, dtype=np.float32).reshape(B, C, N))
    x0 = np.ascontiguousarray(np.asarray(x0, dtype=np.float32).reshape(B, N))
    nc = build_nc()
    in_maps = [
        {"x": x[i * BS : (i + 1) * BS], "x0": x0[i * BS : (i + 1) * BS]}
        for i in range(NCORES)
    ]
    trace = bool(int(os.environ.get("KERNEL_TRACE", "0")))
    res = run_bass_kernel_spmd(nc, in_maps, list(range(NCORES)), trace=trace)
    LAST_RESULTS = res
    pens = np.concatenate([r["pen"].reshape(-1) for r in res.results])
    return np.float32(pens.sum(dtype=np.float64) / B)
